# revision 1
# baseline (speedup 1.0000x reference)
"""Dynamic-weight conv2d (DYDConv2d) Trainium2 kernel.

Problem: per-sample SE-gated mixture of K=4 conv filter banks, then a 3x3
conv (pad 1) with the per-sample aggregated weights.

  pooled = mean_hw(x)                     [B, C]
  h      = relu(pooled @ fc1_w.T)         [B, 65]
  y      = h @ fc2_w.T + fc2_b            [B, 1024]
  prob   = softmax(y.reshape(B,4,256)/30) [B, 4, 256]
  agg    = einsum('bko,kof->bof', prob, W.reshape(4,256,2304))
  out[b] = conv2d(x[b], agg[b].reshape(256,256,3,3), pad=1)

Sharding: pure data-parallel over batch. 8 cores x 2 samples each; every
core holds the full filter bank + SE params. No cross-core comm.

Per-core plan (all conv matmuls bf16, f32 accumulation in PSUM):
 - x loaded f32, cast to a zero-padded bf16 [128, 66, 68] layout per
   ci-block; the cast op also emits the pooled sum (free accum_out).
 - SE chain runs in "transposed" layout so prob lands as per-partition
   scalars: psum_y [128, 8] columns map to (k, o_blk).
 - agg[o, (ci,off)] built on DVE with 1 tensor_scalar + 3 fused
   scalar_tensor_tensor ops per o-block from the pre-cast bf16 W.
 - aggT[ci, off, o] produced by 36 PE transposes (128x128 blocks), copied
   psum->sbuf in batches.
 - conv = 9 shifted matmuls per ci-block accumulating over (ci_blk, off)
   into psum [128, 512] banks; psum->sbuf copy; DMA to HBM.

Emission order is tuned so the serial DMA resource streams
x(s0) -> W(o-blk 0) -> W(o-blk 1) -> x(s1) -> outputs, and the conv for
sample 0 / o-block 0 starts as soon as the first half of W has landed.
"""
import sys

for _p in ("/opt/trn_rl_repo", "/root/.axon_site/_ro/trn_rl_repo"):
    if _p not in sys.path:
        sys.path.insert(0, _p)

import numpy as np

try:  # persistent jax compile cache: makes repeat invocations fast
    import jax
    jax.config.update("jax_compilation_cache_dir", "/tmp/jaxcache")
except Exception:
    pass

import concourse.bass as bass
import concourse.tile as tile
from concourse import bacc, mybir
from concourse.bass_utils import run_bass_kernel_spmd
from concourse.masks import make_identity

F32 = mybir.dt.float32
BF16 = mybir.dt.bfloat16
MULT = mybir.AluOpType.mult
ADD = mybir.AluOpType.add
ACT_COPY = mybir.ActivationFunctionType.Copy
ACT_RELU = mybir.ActivationFunctionType.Relu
ACT_EXP = mybir.ActivationFunctionType.Exp

B, C, H, W = 16, 256, 64, 64
O, K, HID = 256, 4, 65
KK = 3  # kernel spatial size
NOFF = KK * KK  # 9
CF = C * NOFF  # 2304  (ci, off) flattened
N_CORES = 8
BS = B // N_CORES  # samples per core
TEMP = 30.0
# padded x layout: row stride 68 (left pad 2 keeps 4B alignment), 66 rows
PH, PW = H + 2, 68
HWCHUNKS = (1536, 1536, 512, 512)  # free-dim chunking of the 4096 out pixels
TGROUPS = ((0, 4), (4, 8), (8, 9))  # transpose off-batches


def build_kernel(stage=4):
    """stage: 1=through agg, 2=+transposes, 3=+1 conv chunk, 4=full."""
    nc = bacc.Bacc("TRN2", target_bir_lowering=False, debug=False,
                   num_devices=N_CORES)
    x_d = nc.dram_tensor("x", [BS, C, H, W], F32, kind="ExternalInput")
    fc1_d = nc.dram_tensor("fc1_w", [HID, C], F32, kind="ExternalInput")
    fc2_d = nc.dram_tensor("fc2_w", [K * O, HID], F32, kind="ExternalInput")
    fc2b_d = nc.dram_tensor("fc2_b", [K * O], F32, kind="ExternalInput")
    w_d = nc.dram_tensor("weight", [K, O, C, KK, KK], F32, kind="ExternalInput")
    out_d = nc.dram_tensor("out", [BS, O, H, W], F32, kind="ExternalOutput")
    dbg_d = None
    if stage < 3:
        dbg_d = nc.dram_tensor("dbg", [BS, 2, 128, CF], BF16,
                               kind="ExternalOutput")

    with tile.TileContext(nc) as tc:
        _body(nc, tc, x_d, fc1_d, fc2_d, fc2b_d, w_d, out_d, stage, dbg_d)
    nc.compile()
    return nc


def _body(nc, tc, x_d, fc1_d, fc2_d, fc2b_d, w_d, out_d, stage=4, dbg_d=None):
    with (
        tc.tile_pool(name="const", bufs=1) as constp,
        tc.tile_pool(name="wbank", bufs=1) as wbank,
        tc.tile_pool(name="wstage", bufs=5) as wstage,
        tc.tile_pool(name="xf", bufs=2) as xfp,
        tc.tile_pool(name="xb", bufs=1) as xbp,
        tc.tile_pool(name="aggp", bufs=2) as aggp,
        tc.tile_pool(name="aggtp", bufs=2) as aggtp,
        tc.tile_pool(name="small", bufs=2) as smallp,
        tc.tile_pool(name="ost", bufs=3) as ostp,
        tc.tile_pool(name="psc", bufs=2, space=bass.MemorySpace.PSUM) as pscp,
        tc.tile_pool(name="pst", bufs=2, space=bass.MemorySpace.PSUM) as pstp,
    ):
        # ---- params + halo init -----------------------------------------
        # fc1/fc2 are loaded in their natural (contiguous) layouts and
        # transposed on-chip — element-strided gather DMAs are descriptor-
        # bound (~30us for fc2) and would hog the DMA engines at startup.
        with nc.named_scope("params"):
            ident = constp.tile([128, 128], BF16)
            make_identity(nc, ident[:])
            ident32 = constp.tile([128, 128], F32)
            make_identity(nc, ident32[:])
            fc1n = constp.tile([128, C], F32)  # rows 0..64 = fc1_w
            nc.sync.dma_start(fc1n[0:HID, :], fc1_d[:])
            fc2n = constp.tile([128, 8, HID], F32)  # [i_in_blk, i_blk, j]
            nc.sync.dma_start(
                fc2n[:], bass.AP(fc2_d, 0, [[HID, 128], [128 * HID, 8],
                                            [1, HID]]))
            fc1t = constp.tile([128, 2, HID], F32)  # [ci_in_blk, ci_blk, j]
            for blk in range(2):
                tps = pstp.tile([128, HID], F32, tag="pt", name=f"tp1_{blk}")
                nc.tensor.transpose(tps[:], fc1n[0:HID, blk * 128:(blk + 1) * 128],
                                    ident32[0:HID, 0:HID])
                nc.scalar.copy(fc1t[:, blk, :], tps[:])
            fc2t = constp.tile([128, K * O], F32)  # unused rows 66..127
            # rows 0..64 = fc2_w.T ; row 65 = fc2_b (bias folded into matmul)
            for half in range(2):
                tps = pstp.tile([128, 512], F32, tag="pt", name=f"tp2_{half}")
                for c in range(4):
                    nc.tensor.transpose(tps[0:HID, c * 128:(c + 1) * 128],
                                        fc2n[:, half * 4 + c, :], ident32[:])
                nc.vector.tensor_copy(fc2t[0:HID, half * 512:(half + 1) * 512],
                                      tps[0:HID, :])
            nc.sync.dma_start(fc2t[HID:HID + 1, :], fc2b_d[:].unsqueeze(0))
            # zero only the halo cells (full-tile memsets cost ~7.6us each)
            xb = [xbp.tile([128, 2, PH, PW], BF16, name=f"xb{s}")
                  for s in range(BS)]
            for s in range(BS):
                for blk in range(2):
                    nc.gpsimd.memset(xb[s][:, blk, 0, :], 0.0)
                    nc.gpsimd.memset(xb[s][:, blk, PH - 1, :], 0.0)
                    nc.gpsimd.memset(xb[s][:, blk, 0:PH - 1, PW - 2:PW], 0.0)
                    nc.gpsimd.memset(xb[s][:, blk, 1:PH, 0:2], 0.0)

        # ---- x loads + cast/pool ----------------------------------------
        # s0 casts on ACT/DVE (fast, feed the s0 SE chain); s1 casts on the
        # otherwise-idle GPSIMD so they don't block DVE's W-cast/agg chain.
        pooled, se = [], []
        # (ci_blk, pooled col) pairs for the z accumulation, per sample
        zcols = [[(0, 0), (0, 1), (1, 2), (1, 3)], [(0, 0), (1, 1)]]

        def xload_blk(s, blk):
            """s0: two 32-row quarter DMAs + DVE casts (shortens the pooled
            critical path); s1: one full-block DMA + gpsimd cast."""
            with nc.named_scope(f"xload{s}"):
                if blk == 0:
                    pooled.append(smallp.tile([128, 4], F32, tag="pooled",
                                              name=f"pooled{s}"))
                if s > 0:
                    xf = xfp.tile([128, H, W], F32, tag="xf",
                                  name=f"xf{s}_{blk}")
                    nc.sync.dma_start(xf[:], x_d[s, blk * 128:(blk + 1) * 128])
                    interior = xb[s][:, blk, 1:H + 1, 2:W + 2]
                    if blk == 0:
                        nc.scalar.activation(interior, xf[:], ACT_COPY,
                                             accum_out=pooled[s][:, 0:1])
                    else:
                        nc.vector.tensor_scalar(interior, xf[:], 1.0, None,
                                                MULT, ADD,
                                                accum_out=pooled[s][:, 1:2])
                    return
                for hh in range(2):
                    xq = xfp.tile([128, H // 2, W], F32, tag="xq",
                                  name=f"xq{s}_{blk}_{hh}")
                    nc.sync.dma_start(
                        xq[:], x_d[s, blk * 128:(blk + 1) * 128,
                                   hh * 32:(hh + 1) * 32])
                    interior = xb[s][:, blk, 1 + 32 * hh:33 + 32 * hh, 2:W + 2]
                    nc.vector.tensor_scalar(
                        interior, xq[:], 1.0, None, MULT, ADD,
                        accum_out=pooled[s][:, 2 * blk + hh:2 * blk + hh + 1])

        def se_chain(s):
            with nc.named_scope(f"se{s}"):
                z_ps = pstp.tile([128, 1], F32, tag="pt", name=f"z{s}")
                cols = zcols[s]
                for i, (blk, col) in enumerate(cols):
                    nc.tensor.matmul(z_ps[0:HID, :], fc1t[:, blk, :],
                                     pooled[s][:, col:col + 1],
                                     start=(i == 0), stop=(i == len(cols) - 1))
                h_ext = smallp.tile([128, 1], F32, tag="hext", name=f"hext{s}")
                nc.vector.memset(h_ext[:], 1.0)  # row 65 stays 1.0 (bias row)
                # relu(z/4096): mean folded via scale (relu is scale-invariant)
                nc.scalar.activation(h_ext[0:HID, :], z_ps[0:HID, :], ACT_RELU,
                                     scale=1.0 / (H * W))
                y_ps = pstp.tile([128, K * 2], F32, tag="pt", name=f"y{s}")
                for c in range(K * 2):
                    nc.tensor.matmul(y_ps[:, c:c + 1],
                                     fc2t[0:HID + 1, c * 128:(c + 1) * 128],
                                     h_ext[0:HID + 1, :], start=True, stop=True)
                e = smallp.tile([128, K, 2], F32, tag="e", name=f"e{s}")
                nc.scalar.activation(e[:].rearrange("p a b -> p (a b)"),
                                     y_ps[:], ACT_EXP, scale=1.0 / TEMP)
                ssum = smallp.tile([128, 2], F32, tag="ssum", name=f"ssum{s}")
                er = e[:].rearrange("p k o -> p o k")
                nc.vector.tensor_reduce(ssum[:], er, mybir.AxisListType.X, ADD)
                rinv = smallp.tile([128, 2], F32, tag="rinv", name=f"rinv{s}")
                nc.vector.reciprocal(rinv[:], ssum[:])
                prob = smallp.tile([128, 2, K], F32, tag="prob", name=f"prob{s}")
                for ob in range(2):
                    nc.vector.tensor_scalar_mul(prob[:, ob], er[:, ob],
                                                rinv[:, ob:ob + 1])
                return prob

        # DMA queue order: x0, W(ob0), W(ob1), x1, outs
        wb = [wbank.tile([128, K, C, NOFF], BF16, name=f"wb{ob}")
              for ob in range(2)]
        xload_blk(0, 0)
        xload_blk(0, 1)

        def load_w(ob):
            # ci-half-major chunks so agg/transposes for ci-block 0 can
            # start while ci-block 1 is still in flight on the DMA ring
            with nc.named_scope(f"wload{ob}"):
                for cb in range(2):
                    for k in range(K):
                        wst = wstage.tile([128, CF // 2], F32, tag="wst")
                        nc.sync.dma_start(
                            wst[:],
                            w_d[k, ob * 128:(ob + 1) * 128,
                                cb * 128:(cb + 1) * 128].rearrange(
                                    "p c a b -> p (c a b)"))
                        dst = wb[ob][:, k, cb * 128:(cb + 1) * 128, :].rearrange(
                            "p c o -> p (c o)")
                        # all W casts on ACT: DVE owns the x casts + agg
                        # chain at startup and must not self-block
                        nc.scalar.copy(dst, wst[:])

        # agg + transposes for (s, ob), per ci-half -> ob-half of aggt tiles
        def agg_ob(s, ob, agg, aggt):
            for cb in range(2):
                asl = agg[ob][:, cb * 128:(cb + 1) * 128, :]
                with nc.named_scope(f"agg{s}_{ob}"):
                    nc.vector.tensor_scalar_mul(
                        asl, wb[ob][:, 0, cb * 128:(cb + 1) * 128, :],
                        se[s][:, ob, 0:1])
                    for k in range(1, K):
                        nc.vector.scalar_tensor_tensor(
                            asl, wb[ob][:, k, cb * 128:(cb + 1) * 128, :],
                            se[s][:, ob, k:k + 1], asl, MULT, ADD)
                if aggt is None:
                    continue
                with nc.named_scope(f"transp{s}_{ob}"):
                    for gi, (o0, o1) in enumerate(TGROUPS):
                        n = o1 - o0
                        pt = pstp.tile([128, 4, 128], BF16, tag="pt",
                                       name=f"pt{s}_{ob}_{cb}_{gi}")
                        for oi in range(n):
                            nc.tensor.transpose(
                                pt[:, oi, :],
                                agg[ob][:, cb * 128:(cb + 1) * 128, o0 + oi],
                                ident[:])
                        src = pt[:, 0:n, :]
                        dst = aggt[cb][:, o0:o1, ob * 128:(ob + 1) * 128]
                        if (cb * 3 + gi) % 2 == 0:
                            nc.scalar.copy(dst, src)
                        else:
                            nc.vector.tensor_copy(dst, src)

        def conv(s, aggt):
            out_hw = out_d[s].rearrange("o a b -> o (a b)")
            with nc.named_scope(f"conv{s}"):
                for ob in range(2 if stage >= 4 else 1):
                    c0 = 0
                    chunks = HWCHUNKS if stage >= 4 else HWCHUNKS[:1]
                    for ci, csz in enumerate(chunks):
                        pc = pscp.tile([128, max(HWCHUNKS)], F32, tag="conv",
                                       name=f"conv{s}_{ob}_{ci}")
                        for cb in range(2):
                            for off in range(NOFF):
                                dh, dw = off // KK - 1, off % KK - 1
                                lhsT = aggt[cb][:, off, ob * 128:(ob + 1) * 128]
                                for sub in range(csz // 512):
                                    h0 = (c0 + sub * 512) // W
                                    rhs = xb[s][:, cb, h0 + 1 + dh:h0 + 9 + dh,
                                                2 + dw:2 + dw + W]
                                    nc.tensor.matmul(
                                        pc[:, sub * 512:(sub + 1) * 512],
                                        lhsT, rhs,
                                        start=(cb == 0 and off == 0),
                                        stop=(cb == 1 and off == NOFF - 1))
                        ost = ostp.tile([128, max(HWCHUNKS)], F32, tag="ost")
                        if (ob * 3 + ci) % 2 == 0:
                            nc.scalar.copy(ost[:, 0:csz], pc[:, 0:csz])
                        else:
                            nc.vector.tensor_copy(ost[:, 0:csz], pc[:, 0:csz])
                        nc.sync.dma_start(
                            out_hw[ob * 128:(ob + 1) * 128, c0:c0 + csz],
                            ost[:, 0:csz])
                        c0 += csz

        def dbg_dump(s, tiles):
            for i in range(2):
                nc.sync.dma_start(dbg_d[s, i],
                                  tiles[i][:].rearrange("p a b -> p (a b)"))

        # sample 0: interleave with W arrival (ob 0 first)
        agg0 = [aggp.tile([128, C, NOFF], BF16, tag="agg", name=f"agg0_{ob}")
                for ob in range(2)]
        aggt0 = None
        if stage >= 2:
            aggt0 = [aggtp.tile([128, NOFF, O], BF16, tag="aggt",
                                name=f"aggt0_{cb}") for cb in range(2)]
        se.append(se_chain(0))
        load_w(0)
        agg_ob(0, 0, agg0, aggt0)
        load_w(1)
        agg_ob(0, 1, agg0, aggt0)
        xload_blk(1, 0)
        xload_blk(1, 1)
        se.append(se_chain(1))
        if stage == 1:
            dbg_dump(0, agg0)
        elif stage == 2:
            dbg_dump(0, aggt0)
        else:
            conv(0, aggt0)

        # sample 1
        agg1 = [aggp.tile([128, C, NOFF], BF16, tag="agg", name=f"agg1_{ob}")
                for ob in range(2)]
        aggt1 = None
        if stage >= 2:
            aggt1 = [aggtp.tile([128, NOFF, O], BF16, tag="aggt",
                                name=f"aggt1_{cb}") for cb in range(2)]
        for ob in range(2):
            agg_ob(1, ob, agg1, aggt1)
        if stage == 1:
            dbg_dump(1, agg1)
        elif stage == 2:
            dbg_dump(1, aggt1)
        else:
            conv(1, aggt1)


_NC_CACHE = None


def _get_nc():
    global _NC_CACHE
    if _NC_CACHE is None:
        _NC_CACHE = build_kernel()
    return _NC_CACHE


def make_in_maps(x, fc1_w, fc2_w, fc2_b, weight):
    x = np.ascontiguousarray(x, dtype=np.float32)
    shared = {
        "fc1_w": np.ascontiguousarray(fc1_w, dtype=np.float32),
        "fc2_w": np.ascontiguousarray(fc2_w, dtype=np.float32),
        "fc2_b": np.ascontiguousarray(fc2_b, dtype=np.float32),
        "weight": np.ascontiguousarray(weight, dtype=np.float32),
    }
    return [{"x": x[c * BS:(c + 1) * BS], **shared} for c in range(N_CORES)]


def kernel(x, fc1_w, fc2_w, fc2_b, weight):
    import time
    nc = _get_nc()
    in_maps = make_in_maps(x, fc1_w, fc2_w, fc2_b, weight)
    res = None
    for attempt in range(3):
        try:
            res = run_bass_kernel_spmd(nc, in_maps,
                                       core_ids=list(range(N_CORES)))
            break
        except Exception:
            # transient device wedge (NRT_EXEC_UNIT_UNRECOVERABLE); the
            # axon terminal recovers after a short wait
            if attempt == 2:
                raise
            time.sleep(60 * (attempt + 1))
    return np.concatenate([res.results[c]["out"] for c in range(N_CORES)],
                          axis=0).astype(np.float32)



# revision 81
# speedup vs baseline: 1.2359x; 1.2359x over previous
"""Dynamic-weight conv2d (DYDConv2d) Trainium2 kernel — Winograd F(2,3) over H.

Problem: per-sample SE-gated mixture of K=4 conv filter banks, then a 3x3
conv (pad 1) with the per-sample aggregated weights.

  pooled = mean_hw(x)                     [B, C]
  h      = relu(pooled @ fc1_w.T)         [B, 65]
  y      = h @ fc2_w.T + fc2_b            [B, 1024]
  prob   = softmax(y.reshape(B,4,256)/30) [B, 4, 256]
  agg    = einsum('bko,kof->bof', prob, W.reshape(4,256,2304))
  out[b] = conv2d(x[b], agg[b].reshape(256,256,3,3), pad=1)

Sharding: pure data-parallel over batch. 8 cores x 2 samples each; every
core holds the full filter bank + SE params. No cross-core comm.

Per-core plan (conv matmuls bf16, f32 psum accumulation):
 - 1D Winograd F(2,3) along H: row pairs (2t, 2t+1) come from 4 GEMM
   coefficient planes j=0..3 instead of 3 kh taps per row; PE row count
   drops 1.5x (9 -> 6 effective taps per output row pair).
     U0 = d0-d2  U1 = d1+d2  U2 = d2-d1  U3 = d1-d3   (d_m = padded x rows
     m, m+2, .., per 32 tiles; pure DVE tensor_tensor, 2x bf16 mode)
     Wt: j0 = agg[kh=0], j1 = s0+s1+s2, j2 = s0-s1+s2, j3 = agg[kh=2]
     (the F(2,3) 1/2 factor is folded into the PSUM->SBUF copy scale of
     the j1/j2 planes)
     M_j[o,t,w] = sum_{ci,kw} Wt_j[ci,kw,o] U_j[ci,t,w+kw]  (GEMMs)
     out[2t]   = M0+M1+M2;  out[2t+1] = M1-M2-M3            (DVE, writes
     f32 row-interleaved into the DMA staging tile)
 - SE chain identical to the direct kernel: transposed layout so prob
   lands as per-partition scalars.
 - agg mix on DVE (3 slabs) + GPSIMD (1 slab); aggT via 36 PE transposes.
 - M copies: j0/j3 planes ACT plain copy, j1/j2 planes GPSIMD copy with
   scale 0.5.
"""
import sys

for _p in ("/opt/trn_rl_repo", "/root/.axon_site/_ro/trn_rl_repo"):
    if _p not in sys.path:
        sys.path.insert(0, _p)

import numpy as np

try:  # persistent jax compile cache: makes repeat invocations fast
    import jax
    jax.config.update("jax_compilation_cache_dir", "/tmp/jaxcache")
except Exception:
    pass

import concourse.bass as bass
import concourse.tile as tile
from concourse import bacc, mybir
from concourse.bass_utils import run_bass_kernel_spmd
from concourse.masks import make_identity

F32 = mybir.dt.float32
BF16 = mybir.dt.bfloat16
MULT = mybir.AluOpType.mult
ADD = mybir.AluOpType.add
SUB = mybir.AluOpType.subtract
ACT_COPY = mybir.ActivationFunctionType.Copy
ACT_RELU = mybir.ActivationFunctionType.Relu
ACT_EXP = mybir.ActivationFunctionType.Exp

B, C, H, W = 16, 256, 64, 64
O, K, HID = 256, 4, 65
KK = 3  # kernel spatial size
NOFF = KK * KK  # 9
CF = C * NOFF  # 2304  (ci, off) flattened
N_CORES = 8
BS = B // N_CORES  # samples per core
TEMP = 30.0
# padded x layout: row stride 68 (left pad 2 keeps 4B alignment), 66 rows
PH, PW = H + 2, 68
UW = 66  # U width: xb cols 1..66 (covers kw shifts 0..2 over 64 outputs)
NT = H // 2  # 32 winograd row-pair tiles
TCH = 8  # tiles per psum chunk (512 output cols)
TGROUPS = ((0, 4), (4, 8), (8, 9))  # transpose off-batches


def build_kernel(stage=4):
    nc = bacc.Bacc("TRN2", target_bir_lowering=False, debug=False,
                   num_devices=N_CORES)
    x_d = nc.dram_tensor("x", [BS, C, H, W], F32, kind="ExternalInput")
    fc1_d = nc.dram_tensor("fc1_w", [HID, C], F32, kind="ExternalInput")
    fc2_d = nc.dram_tensor("fc2_w", [K * O, HID], F32, kind="ExternalInput")
    fc2b_d = nc.dram_tensor("fc2_b", [K * O], F32, kind="ExternalInput")
    w_d = nc.dram_tensor("weight", [K, O, C, KK, KK], F32, kind="ExternalInput")
    out_d = nc.dram_tensor("out", [BS, O, H, W], F32, kind="ExternalOutput")

    with tile.TileContext(nc) as tc:
        _body(nc, tc, x_d, fc1_d, fc2_d, fc2b_d, w_d, out_d)
    nc.compile()
    return nc


def _body(nc, tc, x_d, fc1_d, fc2_d, fc2b_d, w_d, out_d):
    with (
        tc.tile_pool(name="const", bufs=1) as constp,
        tc.tile_pool(name="wbank", bufs=1) as wbank,
        tc.tile_pool(name="wstage", bufs=3) as wstage,
        tc.tile_pool(name="xf", bufs=4) as xfp,
        tc.tile_pool(name="xb", bufs=2) as xbp,
        tc.tile_pool(name="up", bufs=12) as up,
        tc.tile_pool(name="aggp", bufs=2) as aggp,
        tc.tile_pool(name="aggtp", bufs=2) as aggtp,
        tc.tile_pool(name="wtp", bufs=2) as wtp,
        tc.tile_pool(name="mp", bufs=3) as mp,
        tc.tile_pool(name="invp", bufs=2) as invp,
        tc.tile_pool(name="small", bufs=2) as smallp,
        tc.tile_pool(name="ost", bufs=2) as ostp,
        tc.tile_pool(name="psc", bufs=3, space=bass.MemorySpace.PSUM) as pscp,
        tc.tile_pool(name="pst", bufs=2, space=bass.MemorySpace.PSUM) as pstp,
    ):
        # ---- params ------------------------------------------------------
        # fc1/fc2 are loaded in their natural (contiguous) layouts and
        # transposed on-chip — element-strided gather DMAs are descriptor-
        # bound (~30us for fc2) and would hog the DMA engines at startup.
        # Emitted as a function so the fc DMAs queue after W/x0 startup DMAs.
        prm = {}

        def params():
            with nc.named_scope("params"):
                _params()
            return prm["ident"], prm["fc1t"], prm["fc2t"]

        def _params():
            ident = constp.tile([128, 128], BF16)
            make_identity(nc, ident[:])
            ident32 = constp.tile([128, 128], F32)
            make_identity(nc, ident32[:])
            fc1n = constp.tile([128, C], F32)  # rows 0..64 = fc1_w
            nc.sync.dma_start(fc1n[0:HID, :], fc1_d[:])
            fc2n = constp.tile([128, 8, HID], F32)  # [i_in_blk, i_blk, j]
            nc.sync.dma_start(
                fc2n[:], bass.AP(fc2_d, 0, [[HID, 128], [128 * HID, 8],
                                            [1, HID]]))
            fc1t = constp.tile([128, 2, HID], F32)  # [ci_in_blk, ci_blk, j]
            for blk in range(2):
                tps = pstp.tile([128, HID], F32, tag="pt", name=f"tp1_{blk}")
                nc.tensor.transpose(tps[:], fc1n[0:HID, blk * 128:(blk + 1) * 128],
                                    ident32[0:HID, 0:HID])
                nc.scalar.copy(fc1t[:, blk, :], tps[:])
            fc2t = constp.tile([128, K * O], F32)  # unused rows 66..127
            # rows 0..64 = fc2_w.T ; row 65 = fc2_b (bias folded into matmul)
            for half in range(2):
                tps = pstp.tile([128, 512], F32, tag="pt", name=f"tp2_{half}")
                for c in range(4):
                    nc.tensor.transpose(tps[0:HID, c * 128:(c + 1) * 128],
                                        fc2n[:, half * 4 + c, :], ident32[:])
                nc.scalar.copy(fc2t[0:HID, half * 512:(half + 1) * 512],
                               tps[0:HID, :])
            nc.sync.dma_start(fc2t[HID:HID + 1, :], fc2b_d[:].unsqueeze(0))
            prm.update(ident=ident, fc1t=fc1t, fc2t=fc2t)

        # ---- x loads + cast/pool ----------------------------------------
        # casts on ACT/DVE (split per ci-half) with free pooled accum_out
        pooled, se, xb = [], [], {}
        zcols = [(q // 4, q) for q in range(8)]  # (ci_blk, pooled col)

        xqt = {}

        def xload_dma(s, cb):
            with nc.named_scope(f"xload{s}"):
                if len(pooled) <= s:
                    pooled.append(smallp.tile([128, 8], F32, tag="pooled",
                                              name=f"pooled{s}"))
                t = xbp.tile([128, PH, PW], BF16, tag="xb",
                             name=f"xb{s}_{cb}")
                xb[(s, cb)] = t
                nc.gpsimd.memset(t[:, 0, :], 0.0)
                nc.gpsimd.memset(t[:, PH - 1, :], 0.0)
                nc.gpsimd.memset(t[:, 0:PH - 1, PW - 2:PW], 0.0)
                nc.gpsimd.memset(t[:, 1:PH, 0:2], 0.0)
                for hh in range(4):
                    xq = xfp.tile([128, H // 4, W], F32, tag="xq",
                                  name=f"xq{s}_{cb}_{hh}")
                    xqt[(s, cb, hh)] = xq
                    nc.sync.dma_start(
                        xq[:], x_d[s, cb * 128:(cb + 1) * 128,
                                   hh * 16:(hh + 1) * 16])

        def xcast(s, cb, dve=False):
            t = xb[(s, cb)]
            with nc.named_scope(f"xcast{s}"):
                for hh in range(4):
                    interior = t[:, 1 + 16 * hh:17 + 16 * hh, 2:W + 2]
                    acc = pooled[s][:, 4 * cb + hh:4 * cb + hh + 1]
                    if dve:
                        nc.vector.tensor_scalar(interior, xqt[(s, cb, hh)][:],
                                                1.0, None, MULT, ADD,
                                                accum_out=acc)
                    else:
                        nc.scalar.activation(interior, xqt[(s, cb, hh)][:],
                                             ACT_COPY, accum_out=acc)

        def se_chain(s):
            with nc.named_scope(f"se{s}"):
                z_ps = pstp.tile([128, 1], F32, tag="pt", name=f"z{s}")
                for i, (blk, col) in enumerate(zcols):
                    nc.tensor.matmul(z_ps[0:HID, :], prm["fc1t"][:, blk, :],
                                     pooled[s][:, col:col + 1],
                                     start=(i == 0), stop=(i == len(zcols) - 1))
                h_ext = smallp.tile([128, 1], F32, tag="hext", name=f"hext{s}")
                nc.vector.memset(h_ext[:], 1.0)  # row 65 stays 1.0 (bias row)
                # relu(z/4096): mean folded via scale (relu is scale-invariant)
                nc.scalar.activation(h_ext[0:HID, :], z_ps[0:HID, :], ACT_RELU,
                                     scale=1.0 / (H * W))
                y_ps = pstp.tile([128, K * 2], F32, tag="pt", name=f"y{s}")
                for c in range(K * 2):
                    nc.tensor.matmul(y_ps[:, c:c + 1],
                                     prm["fc2t"][0:HID + 1, c * 128:(c + 1) * 128],
                                     h_ext[0:HID + 1, :], start=True, stop=True)
                e = smallp.tile([128, K, 2], F32, tag="e", name=f"e{s}")
                nc.scalar.activation(e[:].rearrange("p a b -> p (a b)"),
                                     y_ps[:], ACT_EXP, scale=1.0 / TEMP)
                ssum = smallp.tile([128, 2], F32, tag="ssum", name=f"ssum{s}")
                er = e[:].rearrange("p k o -> p o k")
                nc.vector.tensor_reduce(ssum[:], er, mybir.AxisListType.X, ADD)
                rinv = smallp.tile([128, 2], F32, tag="rinv", name=f"rinv{s}")
                nc.vector.reciprocal(rinv[:], ssum[:])
                # softmax denominator folded into a final agg scale (mix_one)
                return e, rinv

        # ---- W load + cast ----------------------------------------------
        wb = [wbank.tile([128, K, C, NOFF], BF16, name=f"wb{ob}")
              for ob in range(2)]

        wstg = {}

        def load_w_dma(ob, cb):
            # ci-half-major chunks so the mix for ci-block 0 can start
            # while ci-block 1 is still in flight on the DMA ring
            with nc.named_scope(f"wload{ob}"):
                for k in range(K):
                    wst = wstage.tile([128, CF // 2], F32, tag="wst")
                    wstg[(ob, cb, k)] = wst
                    nc.sync.dma_start(
                        wst[:],
                        w_d[k, ob * 128:(ob + 1) * 128,
                            cb * 128:(cb + 1) * 128].rearrange(
                                "p c a b -> p (c a b)"))

        def w_cast(ob, cb, dve=False):
            with nc.named_scope(f"wcast{ob}"):
                for k in range(K):
                    dst = wb[ob][:, k, cb * 128:(cb + 1) * 128, :].rearrange(
                        "p c o -> p (c o)")
                    if dve:
                        nc.vector.tensor_copy(dst, wstg[(ob, cb, k)][:])
                    else:
                        nc.scalar.copy(dst, wstg[(ob, cb, k)][:])

        def load_w(ob, cbs=(0, 1)):
            for cb in cbs:
                load_w_dma(ob, cb)
                w_cast(ob, cb)

        # ---- mix + transposes + Wt --------------------------------------
        diag = {}

        def mix_pe(s, ob, agg, cbs):
            """agg[ob] = sum_k diag(e_k) @ W_k on the (startup-idle) PE,
            with the softmax 1/sum folded into the psum->sbuf copy."""
            e, rinv = se[s]
            with nc.named_scope(f"mixpe{s}_{ob}"):
                if (s, ob) not in diag:
                    dg = smallp.tile([128, K, 128], BF16, tag="diag",
                                     name=f"dg{s}_{ob}")
                    for k in range(K):
                        nc.vector.tensor_scalar_mul(dg[:, k, :],
                                                    prm["ident"][:],
                                                    e[:, k, ob:ob + 1])
                    diag[(s, ob)] = dg
                dg = diag[(s, ob)]
                af = agg[ob][:].rearrange("p c o -> p (c o)")
                for cb in cbs:
                    wf = wb[ob][:, :, cb * 128:(cb + 1) * 128, :].rearrange(
                        "p k c o -> p k (c o)")
                    for ci, (c0, cw) in enumerate(
                            ((0, 512), (512, 512), (1024, 128))):
                        ps = pstp.tile([128, 512], F32, tag="pt",
                                       name=f"mx{s}_{ob}_{cb}_{ci}")
                        dst = ps[:, 0:cw]
                        for k in range(K):
                            nc.tensor.matmul(dst, dg[:, k, :],
                                             wf[:, k, c0:c0 + cw],
                                             start=(k == 0), stop=(k == K - 1))
                        nc.scalar.activation(
                            af[:, cb * 1152 + c0:cb * 1152 + c0 + cw], dst,
                            ACT_COPY, scale=rinv[:, ob:ob + 1])

        def mix_one(s, ob, cb, agg, eng=None):
            eng = eng or nc.vector
            e, rinv = se[s]
            asl = agg[ob][:, cb * 128:(cb + 1) * 128, :]
            with nc.named_scope(f"mix{s}_{ob}"):
                eng.tensor_scalar_mul(
                    asl, wb[ob][:, 0, cb * 128:(cb + 1) * 128, :],
                    e[:, 0, ob:ob + 1])
                for k in range(1, K):
                    eng.scalar_tensor_tensor(
                        asl, wb[ob][:, k, cb * 128:(cb + 1) * 128, :],
                        e[:, k, ob:ob + 1], asl, MULT, ADD)
                eng.tensor_scalar_mul(asl, asl, rinv[:, ob:ob + 1])

        def transp(s, ob, agg, aggt, copy_eng=None, cbs=(0, 1)):
            copy = copy_eng or nc.scalar.copy
            with nc.named_scope(f"transp{s}_{ob}"):
                for cb in cbs:
                    for gi, (o0, o1) in enumerate(TGROUPS):
                        n = o1 - o0
                        pt = pstp.tile([128, 4, 128], BF16, tag="pt",
                                       name=f"pt{s}_{ob}_{cb}_{gi}")
                        for oi in range(n):
                            nc.tensor.transpose(
                                pt[:, oi, :],
                                agg[ob][:, cb * 128:(cb + 1) * 128, o0 + oi],
                                prm["ident"][:])
                        src = pt[:, 0:n, :]
                        dst = aggt[cb][:, o0:o1, ob * 128:(ob + 1) * 128]
                        copy(dst, src)

        def wt_build(s, aggt, wt, ob, cbs=(0, 1)):
            """wt[(cb,ob)] = [128, 2, 3, 128]: j1 = s0+s1+s2, j2 = s0-s1+s2
            (kh-planes of aggT); 1/2 factor applied at the M copy."""
            obs = slice(ob * 128, (ob + 1) * 128)
            for cb in cbs:
                t = wtp.tile([128, 2, KK, 128], BF16, tag="wt",
                             name=f"wt{s}_{cb}_{ob}")
                tmp = smallp.tile([128, KK, 128], BF16, tag="wtmp",
                                  name=f"wtmp{s}_{cb}_{ob}")
                a = aggt[cb]
                with nc.named_scope(f"wt{s}"):
                    nc.vector.tensor_tensor(tmp[:], a[:, 0:3, obs],
                                            a[:, 6:9, obs], ADD)
                    nc.vector.tensor_tensor(t[:, 0], tmp[:], a[:, 3:6, obs],
                                            ADD)
                    nc.vector.tensor_tensor(t[:, 1], tmp[:], a[:, 3:6, obs],
                                            SUB)
                wt[(cb, ob)] = t

        # ---- Winograd U build -------------------------------------------
        def u_pair(s, cb, hf, pair, ud):
            """one U pair tile for (s, cb, half): A = (u0, u3), B = (u1, u2);
            [128, 2, NT/2, UW] bf16, cols = xb cols 1..66."""
            t = xb[(s, cb)]

            def d(m):
                r0 = m + 32 * hf
                return t[:, r0:r0 + NT - 1:2, 1:1 + UW]

            nm = "ab"[pair]
            with nc.named_scope(f"u{s}"):
                u = up.tile([128, 2, NT // 2, UW], BF16, tag="u",
                            name=f"u{nm}{s}_{cb}_{hf}")
                if pair == 0:
                    nc.vector.tensor_tensor(u[:, 0], d(0), d(2), SUB)  # u0
                    nc.vector.tensor_tensor(u[:, 1], d(1), d(3), SUB)  # u3
                else:
                    nc.vector.tensor_tensor(u[:, 0], d(1), d(2), ADD)  # u1
                    nc.vector.tensor_tensor(u[:, 1], d(2), d(1), SUB)  # u2
            ud[(cb, hf)] = u

        # ---- conv via winograd GEMMs ------------------------------------
        def conv(s, aggt, wt, ua, ub, fillers, defer=None):
            out_hw = out_d[s].rearrange("o a b -> o (a b)")

            def lhsT(pair, jj, cb, kw, ob):
                obs = slice(ob * 128, (ob + 1) * 128)
                if pair == 0:  # (j0, j3) -> kh plane 0 / 2 of aggT
                    return aggt[cb][:, (0 if jj == 0 else 6) + kw, obs]
                return wt[(cb, ob)][:, jj, kw, :]

            def mms(ps, ob, pair, tc, cb):
                usrc = ua if pair == 0 else ub
                tl = (tc * TCH) % 16
                for jj in range(2):
                    for kw in range(KK):
                        nc.tensor.matmul(
                            ps[:, jj, :],
                            lhsT(pair, jj, cb, kw, ob),
                            usrc[(cb, tc // 2)][:, jj, tl:tl + TCH, kw:kw + W],
                            start=(cb == 0 and kw == 0),
                            stop=(cb == 1 and kw == KK - 1))

            def m_copy(ps, ob, pair, tc, mtile):
                tl = (tc * TCH) % 16
                dst = mtile[:, 2 * pair:2 * pair + 2, tl:tl + TCH, :]
                src = ps[:].rearrange("p a (b c) -> p a b c", b=TCH)
                if pair == 0:
                    nc.scalar.copy(dst, src)
                else:  # fold the F(2,3) 1/2 into the copy
                    nc.scalar.activation(dst, src, ACT_COPY, scale=0.5)

            def m_chunk(ob, pair, tc, mtile):
                ps = pscp.tile([128, 2, 512], F32, tag="conv",
                               name=f"ps{s}_{ob}_{pair}_{tc}")
                for cb in range(2):
                    mms(ps, ob, pair, tc, cb)
                m_copy(ps, ob, pair, tc, mtile)

            def inverse(ob, half, q, mtile, eng=None):
                eng = eng or nc.vector
                # m slots: 0=j0, 1=j3, 2=j1, 3=j2
                # even row 2t   = M0+M1+M2 ; odd row 2t+1 = M1-M2-M3
                st = ostp.tile([128, 16, W], F32, tag="ost",
                               name=f"st{s}_{ob}_{half}_{q}")
                i1 = invp.tile([128, 8, W], BF16, tag="i1",
                               name=f"i1{s}_{ob}_{half}_{q}")
                i2 = invp.tile([128, 8, W], BF16, tag="i2",
                               name=f"i2{s}_{ob}_{half}_{q}")
                tq = slice(8 * q, 8 * q + 8)
                with nc.named_scope(f"inv{s}_{ob}"):
                    eng.tensor_tensor(i1[:], mtile[:, 2, tq],
                                      mtile[:, 3, tq], ADD)
                    eng.tensor_tensor(st[:, 0:16:2, :], i1[:],
                                      mtile[:, 0, tq], ADD)
                    eng.tensor_tensor(i2[:], mtile[:, 2, tq],
                                      mtile[:, 3, tq], SUB)
                    eng.tensor_tensor(st[:, 1:16:2, :], i2[:],
                                      mtile[:, 1, tq], SUB)
                r0 = half * 32 + 16 * q
                nc.sync.dma_start(
                    out_hw[ob * 128:(ob + 1) * 128, r0 * W:(r0 + 16) * W],
                    st[:].rearrange("p a b -> p (a b)"))

            def tail(mtile):
                # final half-block's B chunks split in four 4-tile pieces to
                # shorten the copy -> inverse -> DMA drain after the last mm
                for sub in range(4):
                    ps = pscp.tile([128, 2, 256], F32, tag="conv",
                                   name=f"pstail{sub}")
                    tl = 4 * sub
                    for jj in range(2):
                        for cb in range(2):
                            for kw in range(KK):
                                nc.tensor.matmul(
                                    ps[:, jj, :],
                                    lhsT(1, jj, cb, kw, 1),
                                    ub[(cb, 1)][:, jj, tl:tl + 4, kw:kw + W],
                                    start=(cb == 0 and kw == 0),
                                    stop=(cb == 1 and kw == KK - 1))
                    dst = mtile[:, 2:4, tl:tl + 4, :]
                    nc.scalar.activation(
                        dst, ps[:].rearrange("p a (b c) -> p a b c", b=4),
                        ACT_COPY, scale=0.5)
                    st = ostp.tile([128, 8, W], F32, tag="ost8",
                                   name=f"sttail{sub}")
                    i1 = invp.tile([128, 4, W], BF16, tag="i1",
                                   name=f"i1tail{sub}")
                    i2 = invp.tile([128, 4, W], BF16, tag="i2",
                                   name=f"i2tail{sub}")
                    tq = slice(tl, tl + 4)
                    nc.vector.tensor_tensor(i1[:], mtile[:, 2, tq],
                                            mtile[:, 3, tq], ADD)
                    nc.vector.tensor_tensor(st[:, 0:8:2, :], i1[:],
                                            mtile[:, 0, tq], ADD)
                    nc.vector.tensor_tensor(i2[:], mtile[:, 2, tq],
                                            mtile[:, 3, tq], SUB)
                    nc.vector.tensor_tensor(st[:, 1:8:2, :], i2[:],
                                            mtile[:, 1, tq], SUB)
                    r0 = 32 + 8 * sub
                    nc.sync.dma_start(
                        out_hw[128:256, r0 * W:(r0 + 8) * W],
                        st[:].rearrange("p a b -> p (a b)"))

            def inv_or_defer(ob, hf, q, mtile):
                if defer is not None and ob == 1:
                    defer.append(lambda ob=ob, hf=hf, q=q, m=mtile:
                                 inverse(ob, hf, q, m, eng=nc.gpsimd))
                else:
                    inverse(ob, hf, q, mtile)

            with nc.named_scope(f"conv{s}"):
                for ob in range(2):
                    def point(i, ob=ob):
                        f = fillers.get((ob, point.hf, i))
                        if f is not None:
                            f()
                    mt = [mp.tile([128, 4, 16, W], BF16, tag="m",
                                  name=f"m{s}_{ob}_{hf}") for hf in range(2)]
                    for hf in range(2):
                        point.hf = hf
                        t0, t1 = 2 * hf, 2 * hf + 1
                        if hf == 0:
                            # stream all ci-block-0 matmuls before ci-block
                            # 1's weights/U have finished
                            psa0 = pscp.tile([128, 2, 512], F32, tag="conv",
                                             name=f"psa{s}_{ob}_0")
                            psa1 = pscp.tile([128, 2, 512], F32, tag="conv",
                                             name=f"psa{s}_{ob}_1")
                            psb0 = pscp.tile([128, 2, 512], F32, tag="conv",
                                             name=f"psb{s}_{ob}_0")
                            mms(psa0, ob, 0, t0, 0)
                            point(0)
                            mms(psa1, ob, 0, t1, 0)
                            point(1)
                            mms(psb0, ob, 1, t0, 0)
                            point(2)
                            mms(psa0, ob, 0, t0, 1)
                            m_copy(psa0, ob, 0, t0, mt[hf])
                            point(3)
                            mms(psa1, ob, 0, t1, 1)
                            m_copy(psa1, ob, 0, t1, mt[hf])
                            point(4)
                            mms(psb0, ob, 1, t0, 1)
                            m_copy(psb0, ob, 1, t0, mt[hf])
                            point(5)
                            inv_or_defer(ob, hf, 0, mt[hf])
                            point(6)
                            m_chunk(ob, 1, t1, mt[hf])
                            point(7)
                            inv_or_defer(ob, hf, 1, mt[hf])
                            point(8)
                        else:
                            m_chunk(ob, 0, t0, mt[hf])
                            point(0)
                            m_chunk(ob, 0, t1, mt[hf])
                            point(1)
                            if s == 1 and ob == 1:
                                point(2)
                                point(3)
                                tail(mt[hf])
                                continue
                            m_chunk(ob, 1, t0, mt[hf])
                            point(2)
                            inv_or_defer(ob, hf, 0, mt[hf])
                            point(3)
                            m_chunk(ob, 1, t1, mt[hf])
                            point(4)
                            inv_or_defer(ob, hf, 1, mt[hf])
                            point(5)

        # ---- emission ----------------------------------------------------
        agg0 = [aggp.tile([128, C, NOFF], BF16, tag="agg", name=f"agg0_{ob}")
                for ob in range(2)]
        aggt0 = [aggtp.tile([128, NOFF, O], BF16, tag="aggt",
                            name=f"aggt0_{cb}") for cb in range(2)]
        ua0, ub0, wt0 = {}, {}, {}
        # DMA queue order: W0a, x0c1, x0c0, fc-params, W0b | W1a, W1b,
        # x1c0, x1c1 | conv0 outs
        load_w(0, (0,))
        xload_dma(0, 1)
        xcast(0, 1)
        params()
        xload_dma(0, 0)
        xcast(0, 0)
        se.append(se_chain(0))
        load_w_dma(0, 1)
        w_cast(0, 1)
        u_pair(0, 1, 0, 0, ua0)
        u_pair(0, 1, 0, 1, ub0)
        u_pair(0, 0, 0, 0, ua0)
        mix_pe(0, 0, agg0, (0,))
        u_pair(0, 0, 0, 1, ub0)
        transp(0, 0, agg0, aggt0, cbs=(0,),
               copy_eng=nc.vector.tensor_copy)
        wt_build(0, aggt0, wt0, 0, (0,))
        u_pair(0, 0, 1, 0, ua0)
        u_pair(0, 1, 1, 0, ua0)
        u_pair(0, 0, 1, 1, ub0)
        u_pair(0, 1, 1, 1, ub0)
        # sample-1 DMAs enqueued now; their casts run as conv(0) fillers
        load_w_dma(1, 0)
        load_w_dma(1, 1)
        xload_dma(1, 0)
        xload_dma(1, 1)

        # sample-1 prep emitted as fillers inside conv(0) so the in-order
        # DVE/ACT/PE queues interleave it with sample-0's conv stream;
        # keys are (ob, hf, position) emission points of conv()
        agg1 = [aggp.tile([128, C, NOFF], BF16, tag="agg", name=f"agg1_{ob}")
                for ob in range(2)]
        aggt1 = [aggtp.tile([128, NOFF, O], BF16, tag="aggt",
                            name=f"aggt1_{cb}") for cb in range(2)]
        ua1, ub1, wt1 = {}, {}, {}
        f0 = {
            (0, 0, 2): lambda: (mix_pe(0, 0, agg0, (1,)),
                                transp(0, 0, agg0, aggt0, cbs=(1,),
                                       copy_eng=nc.vector.tensor_copy)),
            (0, 0, 4): lambda: wt_build(0, aggt0, wt0, 0, (1,)),
            (0, 1, 0): lambda: w_cast(1, 0),
            (0, 1, 1): lambda: mix_one(0, 1, 0, agg0),
            (0, 1, 2): lambda: w_cast(1, 1, dve=True),
            (0, 1, 4): lambda: mix_one(0, 1, 1, agg0),
            (0, 1, 5): lambda: transp(0, 1, agg0, aggt0, cbs=(0,),
                                      copy_eng=nc.vector.tensor_copy),
            (1, 0, 0): lambda: wt_build(0, aggt0, wt0, 1, (0,)),
            (1, 0, 1): lambda: xcast(1, 0, dve=True),
            (1, 0, 2): lambda: transp(0, 1, agg0, aggt0, cbs=(1,)),
            (1, 0, 4): lambda: (wt_build(0, aggt0, wt0, 1, (1,)),
                                u_pair(1, 0, 0, 0, ua1),
                                u_pair(1, 0, 0, 1, ub1)),
            (1, 0, 5): lambda: xcast(1, 1, dve=True),
            (1, 0, 6): lambda: se.append(se_chain(1)),
            (1, 1, 0): lambda: (u_pair(1, 1, 0, 0, ua1),
                                u_pair(1, 1, 0, 1, ub1)),
            (1, 1, 1): lambda: (mix_one(1, 0, 0, agg1),
                                mix_one(1, 0, 1, agg1)),
            (1, 1, 3): lambda: (mix_one(1, 1, 0, agg1),
                                mix_one(1, 1, 1, agg1)),
            (1, 1, 5): lambda: (u_pair(1, 0, 1, 0, ua1),
                                u_pair(1, 1, 1, 0, ua1)),
        }
        deferred = []
        conv(0, aggt0, wt0, ua0, ub0, f0, defer=deferred)
        transp(1, 0, agg1, aggt1)
        f1 = {
            (0, 0, 0): lambda: wt_build(1, aggt1, wt1, 0),
            (0, 0, 2): lambda: (u_pair(1, 0, 1, 1, ub1),
                                u_pair(1, 1, 1, 1, ub1)),
            (0, 0, 4): lambda: deferred[0](),
            (0, 0, 6): lambda: deferred[1](),
            (0, 0, 8): lambda: transp(1, 1, agg1, aggt1),
            (0, 1, 0): lambda: wt_build(1, aggt1, wt1, 1),
            (0, 1, 2): lambda: deferred[2](),
            (0, 1, 4): lambda: deferred[3](),
        }
        conv(1, aggt1, wt1, ua1, ub1, f1)


_NC_CACHE = None


def _get_nc():
    global _NC_CACHE
    if _NC_CACHE is None:
        _NC_CACHE = build_kernel()
    return _NC_CACHE


def make_in_maps(x, fc1_w, fc2_w, fc2_b, weight):
    x = np.ascontiguousarray(x, dtype=np.float32)
    shared = {
        "fc1_w": np.ascontiguousarray(fc1_w, dtype=np.float32),
        "fc2_w": np.ascontiguousarray(fc2_w, dtype=np.float32),
        "fc2_b": np.ascontiguousarray(fc2_b, dtype=np.float32),
        "weight": np.ascontiguousarray(weight, dtype=np.float32),
    }
    return [{"x": x[c * BS:(c + 1) * BS], **shared} for c in range(N_CORES)]


def kernel(x, fc1_w, fc2_w, fc2_b, weight):
    import time
    nc = _get_nc()
    in_maps = make_in_maps(x, fc1_w, fc2_w, fc2_b, weight)
    res = None
    for attempt in range(3):
        try:
            res = run_bass_kernel_spmd(nc, in_maps,
                                       core_ids=list(range(N_CORES)))
            break
        except Exception:
            # transient device wedge (NRT_EXEC_UNIT_UNRECOVERABLE); the
            # axon terminal recovers after a short wait
            if attempt == 2:
                raise
            time.sleep(60 * (attempt + 1))
    return np.concatenate([res.results[c]["out"] for c in range(N_CORES)],
                          axis=0).astype(np.float32)


# revision 88
# speedup vs baseline: 1.2560x; 1.0163x over previous
"""Dynamic-weight conv2d (DYDConv2d) Trainium2 kernel — Winograd F(2,3) over H.

Problem: per-sample SE-gated mixture of K=4 conv filter banks, then a 3x3
conv (pad 1) with the per-sample aggregated weights.

  pooled = mean_hw(x)                     [B, C]
  h      = relu(pooled @ fc1_w.T)         [B, 65]
  y      = h @ fc2_w.T + fc2_b            [B, 1024]
  prob   = softmax(y.reshape(B,4,256)/30) [B, 4, 256]
  agg    = einsum('bko,kof->bof', prob, W.reshape(4,256,2304))
  out[b] = conv2d(x[b], agg[b].reshape(256,256,3,3), pad=1)

Sharding: pure data-parallel over batch. 8 cores x 2 samples each; every
core holds the full filter bank + SE params. No cross-core comm.

Per-core plan (conv matmuls bf16, f32 psum accumulation):
 - 1D Winograd F(2,3) along H: row pairs (2t, 2t+1) come from 4 GEMM
   coefficient planes j=0..3 instead of 3 kh taps per row; PE row count
   drops 1.5x (9 -> 6 effective taps per output row pair).
     U0 = d0-d2  U1 = d1+d2  U2 = d2-d1  U3 = d1-d3   (d_m = padded x rows
     m, m+2, .., per 32 tiles; pure DVE tensor_tensor, 2x bf16 mode)
     Wt: j0 = agg[kh=0], j1 = s0+s1+s2, j2 = s0-s1+s2, j3 = agg[kh=2]
     (the F(2,3) 1/2 factor is folded into the PSUM->SBUF copy scale of
     the j1/j2 planes)
     M_j[o,t,w] = sum_{ci,kw} Wt_j[ci,kw,o] U_j[ci,t,w+kw]  (GEMMs)
     out[2t]   = M0+M1+M2;  out[2t+1] = M1-M2-M3            (DVE, writes
     f32 row-interleaved into the DMA staging tile)
 - SE chain in transposed layout so the exp weights land as per-partition
   scalars; the softmax denominator is folded into a final rinv scale so
   the mix can start right after exp (off the reduce/recip latency path).
 - sample-0 agg mix as PE diagonal matmuls (diag(e_k) @ W_k, rinv folded
   into the psum->sbuf copy) — PE is idle during the DMA-bound startup;
   sample-1 mix on DVE as 1 tensor_scalar + 3 scalar_tensor_tensor.
 - aggT via PE transposes (kh-aligned groups); M copies: j0/j3 planes ACT
   plain copy, j1/j2 planes ACT copy with scale 0.5.
 - sample-1 prep (casts, U, mix, transposes) is emitted through a
   point-indexed filler map inside conv(0)'s emission so the in-order
   engine queues interleave it with sample-0's conv stream; sample-0's
   ob1 inverses are deferred into conv(1) to unload DVE in the handoff
   window; the final half-block is split into 4-tile pieces to shorten
   the drain after the last matmul.
"""
import sys

for _p in ("/opt/trn_rl_repo", "/root/.axon_site/_ro/trn_rl_repo"):
    if _p not in sys.path:
        sys.path.insert(0, _p)

import numpy as np

try:  # persistent jax compile cache: makes repeat invocations fast
    import jax
    jax.config.update("jax_compilation_cache_dir", "/tmp/jaxcache")
except Exception:
    pass

import concourse.bass as bass
import concourse.tile as tile
from concourse import bacc, mybir
from concourse.bass_utils import run_bass_kernel_spmd
from concourse.masks import make_identity

F32 = mybir.dt.float32
BF16 = mybir.dt.bfloat16
MULT = mybir.AluOpType.mult
ADD = mybir.AluOpType.add
SUB = mybir.AluOpType.subtract
ACT_COPY = mybir.ActivationFunctionType.Copy
ACT_RELU = mybir.ActivationFunctionType.Relu
ACT_EXP = mybir.ActivationFunctionType.Exp

B, C, H, W = 16, 256, 64, 64
O, K, HID = 256, 4, 65
KK = 3  # kernel spatial size
NOFF = KK * KK  # 9
CF = C * NOFF  # 2304  (ci, off) flattened
N_CORES = 8
BS = B // N_CORES  # samples per core
TEMP = 30.0
# padded x layout: row stride 68 (left pad 2 keeps 4B alignment), 66 rows
PH, PW = H + 2, 68
UW = 66  # U width: xb cols 1..66 (covers kw shifts 0..2 over 64 outputs)
NT = H // 2  # 32 winograd row-pair tiles
TCH = 8  # tiles per psum chunk (512 output cols)
TGROUPS = ((0, 3), (6, 9), (3, 6))  # kh0, kh2 (A-chunk deps) first


def build_kernel(stage=4):
    nc = bacc.Bacc("TRN2", target_bir_lowering=False, debug=False,
                   num_devices=N_CORES)
    x_d = nc.dram_tensor("x", [BS, C, H, W], F32, kind="ExternalInput")
    fc1_d = nc.dram_tensor("fc1_w", [HID, C], F32, kind="ExternalInput")
    fc2_d = nc.dram_tensor("fc2_w", [K * O, HID], F32, kind="ExternalInput")
    fc2b_d = nc.dram_tensor("fc2_b", [K * O], F32, kind="ExternalInput")
    w_d = nc.dram_tensor("weight", [K, O, C, KK, KK], F32, kind="ExternalInput")
    out_d = nc.dram_tensor("out", [BS, O, H, W], F32, kind="ExternalOutput")

    with tile.TileContext(nc) as tc:
        _body(nc, tc, x_d, fc1_d, fc2_d, fc2b_d, w_d, out_d)
    nc.compile()
    return nc


def _body(nc, tc, x_d, fc1_d, fc2_d, fc2b_d, w_d, out_d):
    with (
        tc.tile_pool(name="const", bufs=1) as constp,
        tc.tile_pool(name="wbank", bufs=1) as wbank,
        tc.tile_pool(name="wstage", bufs=3) as wstage,
        tc.tile_pool(name="xf", bufs=4) as xfp,
        tc.tile_pool(name="xb", bufs=2) as xbp,
        tc.tile_pool(name="up", bufs=12) as up,
        tc.tile_pool(name="aggp", bufs=2) as aggp,
        tc.tile_pool(name="aggtp", bufs=2) as aggtp,
        tc.tile_pool(name="wtp", bufs=2) as wtp,
        tc.tile_pool(name="mp", bufs=3) as mp,
        tc.tile_pool(name="invp", bufs=2) as invp,
        tc.tile_pool(name="small", bufs=2) as smallp,
        tc.tile_pool(name="ost", bufs=2) as ostp,
        tc.tile_pool(name="psc", bufs=3, space=bass.MemorySpace.PSUM) as pscp,
        tc.tile_pool(name="pst", bufs=2, space=bass.MemorySpace.PSUM) as pstp,
    ):
        # ---- params ------------------------------------------------------
        # fc1/fc2 are loaded in their natural (contiguous) layouts and
        # transposed on-chip — element-strided gather DMAs are descriptor-
        # bound (~30us for fc2) and would hog the DMA engines at startup.
        # Emitted as a function so the fc DMAs queue after W/x0 startup DMAs.
        prm = {}

        def params():
            with nc.named_scope("params"):
                _params()
            return prm["ident"], prm["fc1t"], prm["fc2t"]

        def _params():
            ident = constp.tile([128, 128], BF16)
            make_identity(nc, ident[:])
            ident32 = constp.tile([128, 128], F32)
            make_identity(nc, ident32[:])
            fc1n = constp.tile([128, C], F32)  # rows 0..64 = fc1_w
            nc.sync.dma_start(fc1n[0:HID, :], fc1_d[:])
            fc2n = constp.tile([128, 8, HID], F32)  # [i_in_blk, i_blk, j]
            nc.sync.dma_start(
                fc2n[:], bass.AP(fc2_d, 0, [[HID, 128], [128 * HID, 8],
                                            [1, HID]]))
            fc1t = constp.tile([128, 2, HID], F32)  # [ci_in_blk, ci_blk, j]
            for blk in range(2):
                tps = pstp.tile([128, HID], F32, tag="pt", name=f"tp1_{blk}")
                nc.tensor.transpose(tps[:], fc1n[0:HID, blk * 128:(blk + 1) * 128],
                                    ident32[0:HID, 0:HID])
                nc.scalar.copy(fc1t[:, blk, :], tps[:])
            fc2t = constp.tile([128, K * O], F32)  # unused rows 66..127
            # rows 0..64 = fc2_w.T ; row 65 = fc2_b (bias folded into matmul)
            for half in range(2):
                tps = pstp.tile([128, 512], F32, tag="pt", name=f"tp2_{half}")
                for c in range(4):
                    nc.tensor.transpose(tps[0:HID, c * 128:(c + 1) * 128],
                                        fc2n[:, half * 4 + c, :], ident32[:])
                nc.scalar.copy(fc2t[0:HID, half * 512:(half + 1) * 512],
                               tps[0:HID, :])
            nc.sync.dma_start(fc2t[HID:HID + 1, :], fc2b_d[:].unsqueeze(0))
            prm.update(ident=ident, fc1t=fc1t, fc2t=fc2t)

        # ---- x loads + cast/pool ----------------------------------------
        # casts on ACT/DVE (split per ci-half) with free pooled accum_out
        pooled, se, xb = [], [], {}
        zcols = [(q // 4, q) for q in range(8)]  # (ci_blk, pooled col)

        xqt = {}

        def xload_dma(s, cb):
            with nc.named_scope(f"xload{s}"):
                if len(pooled) <= s:
                    pooled.append(smallp.tile([128, 8], F32, tag="pooled",
                                              name=f"pooled{s}"))
                t = xbp.tile([128, PH, PW], BF16, tag="xb",
                             name=f"xb{s}_{cb}")
                xb[(s, cb)] = t
                nc.gpsimd.memset(t[:, 0, :], 0.0)
                nc.gpsimd.memset(t[:, PH - 1, :], 0.0)
                nc.gpsimd.memset(t[:, 0:PH - 1, PW - 2:PW], 0.0)
                nc.gpsimd.memset(t[:, 1:PH, 0:2], 0.0)
                for hh in range(4):
                    xq = xfp.tile([128, H // 4, W], F32, tag="xq",
                                  name=f"xq{s}_{cb}_{hh}")
                    xqt[(s, cb, hh)] = xq
                    nc.sync.dma_start(
                        xq[:], x_d[s, cb * 128:(cb + 1) * 128,
                                   hh * 16:(hh + 1) * 16])

        def xcast(s, cb, dve=False):
            t = xb[(s, cb)]
            with nc.named_scope(f"xcast{s}"):
                for hh in range(4):
                    interior = t[:, 1 + 16 * hh:17 + 16 * hh, 2:W + 2]
                    acc = pooled[s][:, 4 * cb + hh:4 * cb + hh + 1]
                    if dve:
                        nc.vector.tensor_scalar(interior, xqt[(s, cb, hh)][:],
                                                1.0, None, MULT, ADD,
                                                accum_out=acc)
                    else:
                        nc.scalar.activation(interior, xqt[(s, cb, hh)][:],
                                             ACT_COPY, accum_out=acc)

        def se_chain(s):
            with nc.named_scope(f"se{s}"):
                z_ps = pstp.tile([128, 1], F32, tag="pt", name=f"z{s}")
                for i, (blk, col) in enumerate(zcols):
                    nc.tensor.matmul(z_ps[0:HID, :], prm["fc1t"][:, blk, :],
                                     pooled[s][:, col:col + 1],
                                     start=(i == 0), stop=(i == len(zcols) - 1))
                h_ext = smallp.tile([128, 1], F32, tag="hext", name=f"hext{s}")
                nc.vector.memset(h_ext[:], 1.0)  # row 65 stays 1.0 (bias row)
                # relu(z/4096): mean folded via scale (relu is scale-invariant)
                nc.scalar.activation(h_ext[0:HID, :], z_ps[0:HID, :], ACT_RELU,
                                     scale=1.0 / (H * W))
                y_ps = pstp.tile([128, K * 2], F32, tag="pt", name=f"y{s}")
                for c in range(K * 2):
                    nc.tensor.matmul(y_ps[:, c:c + 1],
                                     prm["fc2t"][0:HID + 1, c * 128:(c + 1) * 128],
                                     h_ext[0:HID + 1, :], start=True, stop=True)
                e = smallp.tile([128, K, 2], F32, tag="e", name=f"e{s}")
                nc.scalar.activation(e[:].rearrange("p a b -> p (a b)"),
                                     y_ps[:], ACT_EXP, scale=1.0 / TEMP)
                ssum = smallp.tile([128, 2], F32, tag="ssum", name=f"ssum{s}")
                er = e[:].rearrange("p k o -> p o k")
                nc.vector.tensor_reduce(ssum[:], er, mybir.AxisListType.X, ADD)
                rinv = smallp.tile([128, 2], F32, tag="rinv", name=f"rinv{s}")
                nc.vector.reciprocal(rinv[:], ssum[:])
                # softmax denominator folded into a final agg scale (mix_one)
                return e, rinv

        # ---- W load + cast ----------------------------------------------
        wb = [wbank.tile([128, K, C, NOFF], BF16, name=f"wb{ob}")
              for ob in range(2)]

        wstg = {}

        def load_w_dma(ob, cb):
            # ci-half-major chunks so the mix for ci-block 0 can start
            # while ci-block 1 is still in flight on the DMA ring
            with nc.named_scope(f"wload{ob}"):
                for k in range(K):
                    wst = wstage.tile([128, CF // 2], F32, tag="wst")
                    wstg[(ob, cb, k)] = wst
                    nc.sync.dma_start(
                        wst[:],
                        w_d[k, ob * 128:(ob + 1) * 128,
                            cb * 128:(cb + 1) * 128].rearrange(
                                "p c a b -> p (c a b)"))

        def w_cast(ob, cb, dve=False):
            with nc.named_scope(f"wcast{ob}"):
                for k in range(K):
                    dst = wb[ob][:, k, cb * 128:(cb + 1) * 128, :].rearrange(
                        "p c o -> p (c o)")
                    if dve:
                        nc.vector.tensor_copy(dst, wstg[(ob, cb, k)][:])
                    else:
                        nc.scalar.copy(dst, wstg[(ob, cb, k)][:])

        def load_w(ob, cbs=(0, 1)):
            for cb in cbs:
                load_w_dma(ob, cb)
                w_cast(ob, cb)

        # ---- mix + transposes + Wt --------------------------------------
        diag = {}

        def mix_pe(s, ob, agg, cbs):
            """agg[ob] = sum_k diag(e_k) @ W_k on the (startup-idle) PE,
            with the softmax 1/sum folded into the psum->sbuf copy."""
            e, rinv = se[s]
            with nc.named_scope(f"mixpe{s}_{ob}"):
                if (s, ob) not in diag:
                    dg = smallp.tile([128, K, 128], BF16, tag="diag",
                                     name=f"dg{s}_{ob}")
                    for k in range(K):
                        nc.vector.tensor_scalar_mul(dg[:, k, :],
                                                    prm["ident"][:],
                                                    e[:, k, ob:ob + 1])
                    diag[(s, ob)] = dg
                dg = diag[(s, ob)]
                af = agg[ob][:].rearrange("p c o -> p (c o)")
                for cb in cbs:
                    wf = wb[ob][:, :, cb * 128:(cb + 1) * 128, :].rearrange(
                        "p k c o -> p k (c o)")
                    for ci, (c0, cw) in enumerate(
                            ((0, 512), (512, 512), (1024, 128))):
                        ps = pstp.tile([128, 512], F32, tag="pt",
                                       name=f"mx{s}_{ob}_{cb}_{ci}")
                        dst = ps[:, 0:cw]
                        for k in range(K):
                            nc.tensor.matmul(dst, dg[:, k, :],
                                             wf[:, k, c0:c0 + cw],
                                             start=(k == 0), stop=(k == K - 1))
                        nc.scalar.activation(
                            af[:, cb * 1152 + c0:cb * 1152 + c0 + cw], dst,
                            ACT_COPY, scale=rinv[:, ob:ob + 1])

        def mix_one(s, ob, cb, agg, eng=None):
            eng = eng or nc.vector
            e, rinv = se[s]
            asl = agg[ob][:, cb * 128:(cb + 1) * 128, :]
            with nc.named_scope(f"mix{s}_{ob}"):
                eng.tensor_scalar_mul(
                    asl, wb[ob][:, 0, cb * 128:(cb + 1) * 128, :],
                    e[:, 0, ob:ob + 1])
                for k in range(1, K):
                    eng.scalar_tensor_tensor(
                        asl, wb[ob][:, k, cb * 128:(cb + 1) * 128, :],
                        e[:, k, ob:ob + 1], asl, MULT, ADD)
                eng.tensor_scalar_mul(asl, asl, rinv[:, ob:ob + 1])

        def transp(s, ob, agg, aggt, copy_eng=None, cbs=(0, 1)):
            copy = copy_eng or nc.scalar.copy
            with nc.named_scope(f"transp{s}_{ob}"):
                for cb in cbs:
                    for gi, (o0, o1) in enumerate(TGROUPS):
                        n = o1 - o0
                        pt = pstp.tile([128, 4, 128], BF16, tag="pt",
                                       name=f"pt{s}_{ob}_{cb}_{gi}")
                        for oi in range(n):
                            nc.tensor.transpose(
                                pt[:, oi, :],
                                agg[ob][:, cb * 128:(cb + 1) * 128, o0 + oi],
                                prm["ident"][:])
                        src = pt[:, 0:n, :]
                        dst = aggt[cb][:, o0:o1, ob * 128:(ob + 1) * 128]
                        copy(dst, src)

        def wt_build(s, aggt, wt, ob, cbs=(0, 1)):
            """wt[(cb,ob)] = [128, 2, 3, 128]: j1 = s0+s1+s2, j2 = s0-s1+s2
            (kh-planes of aggT); 1/2 factor applied at the M copy."""
            obs = slice(ob * 128, (ob + 1) * 128)
            for cb in cbs:
                t = wtp.tile([128, 2, KK, 128], BF16, tag="wt",
                             name=f"wt{s}_{cb}_{ob}")
                tmp = smallp.tile([128, KK, 128], BF16, tag="wtmp",
                                  name=f"wtmp{s}_{cb}_{ob}")
                a = aggt[cb]
                with nc.named_scope(f"wt{s}"):
                    nc.vector.tensor_tensor(tmp[:], a[:, 0:3, obs],
                                            a[:, 6:9, obs], ADD)
                    nc.vector.tensor_tensor(t[:, 0], tmp[:], a[:, 3:6, obs],
                                            ADD)
                    nc.vector.tensor_tensor(t[:, 1], tmp[:], a[:, 3:6, obs],
                                            SUB)
                wt[(cb, ob)] = t

        # ---- Winograd U build -------------------------------------------
        def u_pair(s, cb, hf, pair, ud):
            """one U pair tile for (s, cb, half): A = (u0, u3), B = (u1, u2);
            [128, 2, NT/2, UW] bf16, cols = xb cols 1..66."""
            t = xb[(s, cb)]

            def d(m):
                r0 = m + 32 * hf
                return t[:, r0:r0 + NT - 1:2, 1:1 + UW]

            nm = "ab"[pair]
            with nc.named_scope(f"u{s}"):
                u = up.tile([128, 2, NT // 2, UW], BF16, tag="u",
                            name=f"u{nm}{s}_{cb}_{hf}")
                if pair == 0:
                    nc.vector.tensor_tensor(u[:, 0], d(0), d(2), SUB)  # u0
                    nc.vector.tensor_tensor(u[:, 1], d(1), d(3), SUB)  # u3
                else:
                    nc.vector.tensor_tensor(u[:, 0], d(1), d(2), ADD)  # u1
                    nc.vector.tensor_tensor(u[:, 1], d(2), d(1), SUB)  # u2
            ud[(cb, hf)] = u

        # ---- conv via winograd GEMMs ------------------------------------
        def conv(s, aggt, wt, ua, ub, fillers, defer=None):
            out_hw = out_d[s].rearrange("o a b -> o (a b)")

            def lhsT(pair, jj, cb, kw, ob):
                obs = slice(ob * 128, (ob + 1) * 128)
                if pair == 0:  # (j0, j3) -> kh plane 0 / 2 of aggT
                    return aggt[cb][:, (0 if jj == 0 else 6) + kw, obs]
                return wt[(cb, ob)][:, jj, kw, :]

            def mms(ps, ob, pair, tc, cb):
                usrc = ua if pair == 0 else ub
                tl = (tc * TCH) % 16
                for jj in range(2):
                    for kw in range(KK):
                        nc.tensor.matmul(
                            ps[:, jj, :],
                            lhsT(pair, jj, cb, kw, ob),
                            usrc[(cb, tc // 2)][:, jj, tl:tl + TCH, kw:kw + W],
                            start=(cb == 0 and kw == 0),
                            stop=(cb == 1 and kw == KK - 1))

            def m_copy(ps, ob, pair, tc, mtile):
                tl = (tc * TCH) % 16
                dst = mtile[:, 2 * pair:2 * pair + 2, tl:tl + TCH, :]
                src = ps[:].rearrange("p a (b c) -> p a b c", b=TCH)
                if pair == 0:
                    nc.scalar.copy(dst, src)
                else:  # fold the F(2,3) 1/2 into the copy
                    nc.scalar.activation(dst, src, ACT_COPY, scale=0.5)

            def m_chunk(ob, pair, tc, mtile):
                ps = pscp.tile([128, 2, 512], F32, tag="conv",
                               name=f"ps{s}_{ob}_{pair}_{tc}")
                for cb in range(2):
                    mms(ps, ob, pair, tc, cb)
                m_copy(ps, ob, pair, tc, mtile)

            def inverse(ob, half, q, mtile, eng=None):
                eng = eng or nc.vector
                # m slots: 0=j0, 1=j3, 2=j1, 3=j2
                # even row 2t   = M0+M1+M2 ; odd row 2t+1 = M1-M2-M3
                st = ostp.tile([128, 16, W], F32, tag="ost", bufs=2,
                               name=f"st{s}_{ob}_{half}_{q}")
                i1 = invp.tile([128, 8, W], BF16, tag="i1",
                               name=f"i1{s}_{ob}_{half}_{q}")
                i2 = invp.tile([128, 8, W], BF16, tag="i2",
                               name=f"i2{s}_{ob}_{half}_{q}")
                tq = slice(8 * q, 8 * q + 8)
                with nc.named_scope(f"inv{s}_{ob}"):
                    eng.tensor_tensor(i1[:], mtile[:, 2, tq],
                                      mtile[:, 3, tq], ADD)
                    eng.tensor_tensor(st[:, 0:16:2, :], i1[:],
                                      mtile[:, 0, tq], ADD)
                    eng.tensor_tensor(i2[:], mtile[:, 2, tq],
                                      mtile[:, 3, tq], SUB)
                    eng.tensor_tensor(st[:, 1:16:2, :], i2[:],
                                      mtile[:, 1, tq], SUB)
                r0 = half * 32 + 16 * q
                nc.sync.dma_start(
                    out_hw[ob * 128:(ob + 1) * 128, r0 * W:(r0 + 16) * W],
                    st[:].rearrange("p a b -> p (a b)"))

            def tail(mtile):
                # final half-block's B chunks split in four 4-tile pieces to
                # shorten the copy -> inverse -> DMA drain after the last mm
                for sub in range(4):
                    ps = pscp.tile([128, 2, 256], F32, tag="conv",
                                   name=f"pstail{sub}")
                    tl = 4 * sub
                    for jj in range(2):
                        for cb in range(2):
                            for kw in range(KK):
                                nc.tensor.matmul(
                                    ps[:, jj, :],
                                    lhsT(1, jj, cb, kw, 1),
                                    ub[(cb, 1)][:, jj, tl:tl + 4, kw:kw + W],
                                    start=(cb == 0 and kw == 0),
                                    stop=(cb == 1 and kw == KK - 1))
                    dst = mtile[:, 2:4, tl:tl + 4, :]
                    nc.scalar.activation(
                        dst, ps[:].rearrange("p a (b c) -> p a b c", b=4),
                        ACT_COPY, scale=0.5)
                    st = ostp.tile([128, 8, W], F32, tag="ost8", bufs=3,
                                   name=f"sttail{sub}")
                    i1 = invp.tile([128, 4, W], BF16, tag="i1",
                                   name=f"i1tail{sub}")
                    i2 = invp.tile([128, 4, W], BF16, tag="i2",
                                   name=f"i2tail{sub}")
                    tq = slice(tl, tl + 4)
                    nc.vector.tensor_tensor(i1[:], mtile[:, 2, tq],
                                            mtile[:, 3, tq], ADD)
                    nc.vector.tensor_tensor(st[:, 0:8:2, :], i1[:],
                                            mtile[:, 0, tq], ADD)
                    nc.vector.tensor_tensor(i2[:], mtile[:, 2, tq],
                                            mtile[:, 3, tq], SUB)
                    nc.vector.tensor_tensor(st[:, 1:8:2, :], i2[:],
                                            mtile[:, 1, tq], SUB)
                    r0 = 32 + 8 * sub
                    nc.sync.dma_start(
                        out_hw[128:256, r0 * W:(r0 + 8) * W],
                        st[:].rearrange("p a b -> p (a b)"))

            def inv_or_defer(ob, hf, q, mtile):
                if defer is not None and ob == 1:
                    defer.append(lambda ob=ob, hf=hf, q=q, m=mtile:
                                 inverse(ob, hf, q, m, eng=nc.gpsimd))
                else:
                    inverse(ob, hf, q, mtile)

            with nc.named_scope(f"conv{s}"):
                for ob in range(2):
                    def point(i, ob=ob):
                        f = fillers.get((ob, point.hf, i))
                        if f is not None:
                            f()
                    mt = [mp.tile([128, 4, 16, W], BF16, tag="m",
                                  name=f"m{s}_{ob}_{hf}") for hf in range(2)]
                    for hf in range(2):
                        point.hf = hf
                        t0, t1 = 2 * hf, 2 * hf + 1
                        if hf == 0:
                            # stream all ci-block-0 matmuls before ci-block
                            # 1's weights/U have finished
                            psa0 = pscp.tile([128, 2, 512], F32, tag="conv",
                                             name=f"psa{s}_{ob}_0")
                            psa1 = pscp.tile([128, 2, 512], F32, tag="conv",
                                             name=f"psa{s}_{ob}_1")
                            psb0 = pscp.tile([128, 2, 512], F32, tag="conv",
                                             name=f"psb{s}_{ob}_0")
                            mms(psa0, ob, 0, t0, 0)
                            point(0)
                            mms(psa1, ob, 0, t1, 0)
                            point(1)
                            mms(psb0, ob, 1, t0, 0)
                            point(2)
                            mms(psa0, ob, 0, t0, 1)
                            m_copy(psa0, ob, 0, t0, mt[hf])
                            point(3)
                            mms(psa1, ob, 0, t1, 1)
                            m_copy(psa1, ob, 0, t1, mt[hf])
                            point(4)
                            mms(psb0, ob, 1, t0, 1)
                            m_copy(psb0, ob, 1, t0, mt[hf])
                            point(5)
                            inv_or_defer(ob, hf, 0, mt[hf])
                            point(6)
                            m_chunk(ob, 1, t1, mt[hf])
                            point(7)
                            inv_or_defer(ob, hf, 1, mt[hf])
                            point(8)
                        else:
                            m_chunk(ob, 0, t0, mt[hf])
                            point(0)
                            m_chunk(ob, 0, t1, mt[hf])
                            point(1)
                            if s == 1 and ob == 1:
                                point(2)
                                point(3)
                                tail(mt[hf])
                                continue
                            m_chunk(ob, 1, t0, mt[hf])
                            point(2)
                            inv_or_defer(ob, hf, 0, mt[hf])
                            point(3)
                            m_chunk(ob, 1, t1, mt[hf])
                            point(4)
                            inv_or_defer(ob, hf, 1, mt[hf])
                            point(5)

        # ---- emission ----------------------------------------------------
        agg0 = [aggp.tile([128, C, NOFF], BF16, tag="agg", name=f"agg0_{ob}")
                for ob in range(2)]
        aggt0 = [aggtp.tile([128, NOFF, O], BF16, tag="aggt",
                            name=f"aggt0_{cb}") for cb in range(2)]
        ua0, ub0, wt0 = {}, {}, {}
        # DMA queue order: W0a, x0c1, x0c0, fc-params, W0b | W1a, W1b,
        # x1c0, x1c1 | conv0 outs
        load_w(0, (0,))
        xload_dma(0, 1)
        xcast(0, 1)
        params()
        xload_dma(0, 0)
        xcast(0, 0)
        se.append(se_chain(0))
        load_w_dma(0, 1)
        w_cast(0, 1)
        u_pair(0, 1, 0, 0, ua0)
        u_pair(0, 1, 0, 1, ub0)
        u_pair(0, 0, 0, 0, ua0)
        mix_pe(0, 0, agg0, (0,))
        u_pair(0, 0, 0, 1, ub0)
        transp(0, 0, agg0, aggt0, cbs=(0,),
               copy_eng=nc.vector.tensor_copy)
        wt_build(0, aggt0, wt0, 0, (0,))
        u_pair(0, 0, 1, 0, ua0)
        u_pair(0, 1, 1, 0, ua0)
        u_pair(0, 0, 1, 1, ub0)
        u_pair(0, 1, 1, 1, ub0)
        # sample-1 DMAs enqueued now; their casts run as conv(0) fillers
        load_w_dma(1, 0)
        load_w_dma(1, 1)
        xload_dma(1, 0)
        xload_dma(1, 1)

        # sample-1 prep emitted as fillers inside conv(0) so the in-order
        # DVE/ACT/PE queues interleave it with sample-0's conv stream;
        # keys are (ob, hf, position) emission points of conv()
        agg1 = [aggp.tile([128, C, NOFF], BF16, tag="agg", name=f"agg1_{ob}")
                for ob in range(2)]
        aggt1 = [aggtp.tile([128, NOFF, O], BF16, tag="aggt",
                            name=f"aggt1_{cb}") for cb in range(2)]
        ua1, ub1, wt1 = {}, {}, {}
        f0 = {
            (0, 0, 2): lambda: (mix_pe(0, 0, agg0, (1,)),
                                transp(0, 0, agg0, aggt0, cbs=(1,),
                                       copy_eng=nc.vector.tensor_copy)),
            (0, 0, 4): lambda: wt_build(0, aggt0, wt0, 0, (1,)),
            (0, 1, 0): lambda: w_cast(1, 0),
            (0, 1, 1): lambda: mix_one(0, 1, 0, agg0),
            (0, 1, 2): lambda: w_cast(1, 1, dve=True),
            (0, 1, 4): lambda: mix_one(0, 1, 1, agg0),
            (0, 1, 5): lambda: transp(0, 1, agg0, aggt0, cbs=(0,)),
            (1, 0, 0): lambda: wt_build(0, aggt0, wt0, 1, (0,)),
            (1, 0, 1): lambda: xcast(1, 0, dve=True),
            (1, 0, 2): lambda: transp(0, 1, agg0, aggt0, cbs=(1,)),
            (1, 0, 4): lambda: (wt_build(0, aggt0, wt0, 1, (1,)),
                                u_pair(1, 0, 0, 0, ua1),
                                u_pair(1, 0, 0, 1, ub1)),
            (1, 0, 5): lambda: xcast(1, 1, dve=True),
            (1, 0, 6): lambda: se.append(se_chain(1)),
            (1, 1, 0): lambda: (u_pair(1, 1, 0, 0, ua1),
                                u_pair(1, 1, 0, 1, ub1)),
            (1, 1, 1): lambda: (mix_one(1, 0, 0, agg1),
                                mix_one(1, 0, 1, agg1)),
            (1, 1, 3): lambda: (mix_one(1, 1, 0, agg1),
                                mix_one(1, 1, 1, agg1)),
            (1, 1, 5): lambda: (u_pair(1, 0, 1, 0, ua1),
                                u_pair(1, 1, 1, 0, ua1)),
        }
        deferred = []
        conv(0, aggt0, wt0, ua0, ub0, f0, defer=deferred)
        transp(1, 0, agg1, aggt1)
        f1 = {
            (0, 0, 0): lambda: wt_build(1, aggt1, wt1, 0),
            (0, 0, 2): lambda: (u_pair(1, 0, 1, 1, ub1),
                                u_pair(1, 1, 1, 1, ub1)),
            (0, 0, 4): lambda: deferred[0](),
            (0, 0, 6): lambda: deferred[1](),
            (0, 0, 8): lambda: transp(1, 1, agg1, aggt1),
            (0, 1, 0): lambda: wt_build(1, aggt1, wt1, 1),
            (0, 1, 2): lambda: deferred[2](),
            (0, 1, 4): lambda: deferred[3](),
        }
        conv(1, aggt1, wt1, ua1, ub1, f1)


_NC_CACHE = None


def _get_nc():
    global _NC_CACHE
    if _NC_CACHE is None:
        _NC_CACHE = build_kernel()
    return _NC_CACHE


def make_in_maps(x, fc1_w, fc2_w, fc2_b, weight):
    x = np.ascontiguousarray(x, dtype=np.float32)
    shared = {
        "fc1_w": np.ascontiguousarray(fc1_w, dtype=np.float32),
        "fc2_w": np.ascontiguousarray(fc2_w, dtype=np.float32),
        "fc2_b": np.ascontiguousarray(fc2_b, dtype=np.float32),
        "weight": np.ascontiguousarray(weight, dtype=np.float32),
    }
    return [{"x": x[c * BS:(c + 1) * BS], **shared} for c in range(N_CORES)]


def kernel(x, fc1_w, fc2_w, fc2_b, weight):
    import time
    nc = _get_nc()
    in_maps = make_in_maps(x, fc1_w, fc2_w, fc2_b, weight)
    res = None
    for attempt in range(3):
        try:
            res = run_bass_kernel_spmd(nc, in_maps,
                                       core_ids=list(range(N_CORES)))
            break
        except Exception:
            # transient device wedge (NRT_EXEC_UNIT_UNRECOVERABLE); the
            # axon terminal recovers after a short wait
            if attempt == 2:
                raise
            time.sleep(60 * (attempt + 1))
    return np.concatenate([res.results[c]["out"] for c in range(N_CORES)],
                          axis=0).astype(np.float32)


# revision 89
# speedup vs baseline: 1.2749x; 1.0150x over previous
"""Dynamic-weight conv2d (DYDConv2d) Trainium2 kernel — Winograd F(2,3) over H.

Problem: per-sample SE-gated mixture of K=4 conv filter banks, then a 3x3
conv (pad 1) with the per-sample aggregated weights.

  pooled = mean_hw(x)                     [B, C]
  h      = relu(pooled @ fc1_w.T)         [B, 65]
  y      = h @ fc2_w.T + fc2_b            [B, 1024]
  prob   = softmax(y.reshape(B,4,256)/30) [B, 4, 256]
  agg    = einsum('bko,kof->bof', prob, W.reshape(4,256,2304))
  out[b] = conv2d(x[b], agg[b].reshape(256,256,3,3), pad=1)

Sharding: pure data-parallel over batch. 8 cores x 2 samples each; every
core holds the full filter bank + SE params. No cross-core comm.

Per-core plan (conv matmuls bf16, f32 psum accumulation):
 - 1D Winograd F(2,3) along H: row pairs (2t, 2t+1) come from 4 GEMM
   coefficient planes j=0..3 instead of 3 kh taps per row; PE row count
   drops 1.5x (9 -> 6 effective taps per output row pair).
     U0 = d0-d2  U1 = d1+d2  U2 = d2-d1  U3 = d1-d3   (d_m = padded x rows
     m, m+2, .., per 32 tiles; pure DVE tensor_tensor, 2x bf16 mode)
     Wt: j0 = agg[kh=0], j1 = s0+s1+s2, j2 = s0-s1+s2, j3 = agg[kh=2]
     (the F(2,3) 1/2 factor is folded into the PSUM->SBUF copy scale of
     the j1/j2 planes)
     M_j[o,t,w] = sum_{ci,kw} Wt_j[ci,kw,o] U_j[ci,t,w+kw]  (GEMMs)
     out[2t]   = M0+M1+M2;  out[2t+1] = M1-M2-M3            (DVE, writes
     f32 row-interleaved into the DMA staging tile)
 - SE chain in transposed layout so the exp weights land as per-partition
   scalars; the softmax denominator is folded into a final rinv scale so
   the mix can start right after exp (off the reduce/recip latency path).
 - sample-0 agg mix as PE diagonal matmuls (diag(e_k) @ W_k, rinv folded
   into the psum->sbuf copy) — PE is idle during the DMA-bound startup;
   sample-1 mix on DVE as 1 tensor_scalar + 3 scalar_tensor_tensor.
 - aggT via PE transposes (kh-aligned groups); M copies: j0/j3 planes ACT
   plain copy, j1/j2 planes ACT copy with scale 0.5.
 - sample-1 prep (casts, U, mix, transposes) is emitted through a
   point-indexed filler map inside conv(0)'s emission so the in-order
   engine queues interleave it with sample-0's conv stream; sample-0's
   ob1 inverses are deferred into conv(1) to unload DVE in the handoff
   window; the final half-block is split into 4-tile pieces to shorten
   the drain after the last matmul.
"""
import sys

for _p in ("/opt/trn_rl_repo", "/root/.axon_site/_ro/trn_rl_repo"):
    if _p not in sys.path:
        sys.path.insert(0, _p)

import numpy as np

try:  # persistent jax compile cache: makes repeat invocations fast
    import jax
    jax.config.update("jax_compilation_cache_dir", "/tmp/jaxcache")
except Exception:
    pass

import concourse.bass as bass
import concourse.tile as tile
from concourse import bacc, mybir
from concourse.bass_utils import run_bass_kernel_spmd
from concourse.masks import make_identity

F32 = mybir.dt.float32
BF16 = mybir.dt.bfloat16
MULT = mybir.AluOpType.mult
ADD = mybir.AluOpType.add
SUB = mybir.AluOpType.subtract
ACT_COPY = mybir.ActivationFunctionType.Copy
ACT_RELU = mybir.ActivationFunctionType.Relu
ACT_EXP = mybir.ActivationFunctionType.Exp

B, C, H, W = 16, 256, 64, 64
O, K, HID = 256, 4, 65
KK = 3  # kernel spatial size
NOFF = KK * KK  # 9
CF = C * NOFF  # 2304  (ci, off) flattened
N_CORES = 8
BS = B // N_CORES  # samples per core
TEMP = 30.0
# padded x layout: row stride 68 (left pad 2 keeps 4B alignment), 66 rows
PH, PW = H + 2, 68
UW = 66  # U width: xb cols 1..66 (covers kw shifts 0..2 over 64 outputs)
NT = H // 2  # 32 winograd row-pair tiles
TCH = 8  # tiles per psum chunk (512 output cols)
TGROUPS = ((0, 3), (6, 9), (3, 6))  # kh0, kh2 (A-chunk deps) first


def build_kernel(stage=4):
    nc = bacc.Bacc("TRN2", target_bir_lowering=False, debug=False,
                   num_devices=N_CORES)
    x_d = nc.dram_tensor("x", [BS, C, H, W], F32, kind="ExternalInput")
    fc1_d = nc.dram_tensor("fc1_w", [HID, C], F32, kind="ExternalInput")
    fc2_d = nc.dram_tensor("fc2_w", [K * O, HID], F32, kind="ExternalInput")
    fc2b_d = nc.dram_tensor("fc2_b", [K * O], F32, kind="ExternalInput")
    w_d = nc.dram_tensor("weight", [K, O, C, KK, KK], F32, kind="ExternalInput")
    out_d = nc.dram_tensor("out", [BS, O, H, W], F32, kind="ExternalOutput")

    with tile.TileContext(nc) as tc:
        _body(nc, tc, x_d, fc1_d, fc2_d, fc2b_d, w_d, out_d)
    nc.compile()
    return nc


def _body(nc, tc, x_d, fc1_d, fc2_d, fc2b_d, w_d, out_d):
    with (
        tc.tile_pool(name="const", bufs=1) as constp,
        tc.tile_pool(name="wbank", bufs=1) as wbank,
        tc.tile_pool(name="wstage", bufs=3) as wstage,
        tc.tile_pool(name="xf", bufs=4) as xfp,
        tc.tile_pool(name="xb", bufs=2) as xbp,
        tc.tile_pool(name="up", bufs=12) as up,
        tc.tile_pool(name="aggp", bufs=2) as aggp,
        tc.tile_pool(name="aggtp", bufs=2) as aggtp,
        tc.tile_pool(name="wtp", bufs=2) as wtp,
        tc.tile_pool(name="mp", bufs=3) as mp,
        tc.tile_pool(name="invp", bufs=2) as invp,
        tc.tile_pool(name="small", bufs=2) as smallp,
        tc.tile_pool(name="ost", bufs=2) as ostp,
        tc.tile_pool(name="psc", bufs=3, space=bass.MemorySpace.PSUM) as pscp,
        tc.tile_pool(name="pst", bufs=2, space=bass.MemorySpace.PSUM) as pstp,
    ):
        # ---- params ------------------------------------------------------
        # fc1/fc2 are loaded in their natural (contiguous) layouts and
        # transposed on-chip — element-strided gather DMAs are descriptor-
        # bound (~30us for fc2) and would hog the DMA engines at startup.
        # Emitted as a function so the fc DMAs queue after W/x0 startup DMAs.
        prm = {}

        def params():
            with nc.named_scope("params"):
                _params()
            return prm["ident"], prm["fc1t"], prm["fc2t"]

        def _params():
            ident = constp.tile([128, 128], BF16)
            make_identity(nc, ident[:])
            ident32 = constp.tile([128, 128], F32)
            make_identity(nc, ident32[:])
            fc1n = constp.tile([128, C], F32)  # rows 0..64 = fc1_w
            nc.sync.dma_start(fc1n[0:HID, :], fc1_d[:])
            fc2n = constp.tile([128, 8, HID], F32)  # [i_in_blk, i_blk, j]
            nc.sync.dma_start(
                fc2n[:], bass.AP(fc2_d, 0, [[HID, 128], [128 * HID, 8],
                                            [1, HID]]))
            fc1t = constp.tile([128, 2, HID], F32)  # [ci_in_blk, ci_blk, j]
            for blk in range(2):
                tps = pstp.tile([128, HID], F32, tag="pt", name=f"tp1_{blk}")
                nc.tensor.transpose(tps[:], fc1n[0:HID, blk * 128:(blk + 1) * 128],
                                    ident32[0:HID, 0:HID])
                nc.scalar.copy(fc1t[:, blk, :], tps[:])
            fc2t = constp.tile([128, K * O], F32)  # unused rows 66..127
            # rows 0..64 = fc2_w.T ; row 65 = fc2_b (bias folded into matmul)
            for half in range(2):
                tps = pstp.tile([128, 512], F32, tag="pt", name=f"tp2_{half}")
                for c in range(4):
                    nc.tensor.transpose(tps[0:HID, c * 128:(c + 1) * 128],
                                        fc2n[:, half * 4 + c, :], ident32[:])
                nc.scalar.copy(fc2t[0:HID, half * 512:(half + 1) * 512],
                               tps[0:HID, :])
            nc.sync.dma_start(fc2t[HID:HID + 1, :], fc2b_d[:].unsqueeze(0))
            prm.update(ident=ident, fc1t=fc1t, fc2t=fc2t)

        # ---- x loads + cast/pool ----------------------------------------
        # casts on ACT/DVE (split per ci-half) with free pooled accum_out
        pooled, se, xb = [], [], {}
        zcols = [(q // 4, q) for q in range(8)]  # (ci_blk, pooled col)

        xqt = {}

        def xload_dma(s, cb):
            with nc.named_scope(f"xload{s}"):
                if len(pooled) <= s:
                    pooled.append(smallp.tile([128, 8], F32, tag="pooled",
                                              name=f"pooled{s}"))
                t = xbp.tile([128, PH, PW], BF16, tag="xb",
                             name=f"xb{s}_{cb}")
                xb[(s, cb)] = t
                nc.gpsimd.memset(t[:, 0, :], 0.0)
                nc.gpsimd.memset(t[:, PH - 1, :], 0.0)
                nc.gpsimd.memset(t[:, 0:PH - 1, PW - 2:PW], 0.0)
                nc.gpsimd.memset(t[:, 1:PH, 0:2], 0.0)
                for hh in range(4):
                    xq = xfp.tile([128, H // 4, W], F32, tag="xq",
                                  name=f"xq{s}_{cb}_{hh}")
                    xqt[(s, cb, hh)] = xq
                    nc.sync.dma_start(
                        xq[:], x_d[s, cb * 128:(cb + 1) * 128,
                                   hh * 16:(hh + 1) * 16])

        def xcast(s, cb, dve=False):
            t = xb[(s, cb)]
            with nc.named_scope(f"xcast{s}"):
                for hh in range(4):
                    interior = t[:, 1 + 16 * hh:17 + 16 * hh, 2:W + 2]
                    acc = pooled[s][:, 4 * cb + hh:4 * cb + hh + 1]
                    if dve:
                        nc.vector.tensor_scalar(interior, xqt[(s, cb, hh)][:],
                                                1.0, None, MULT, ADD,
                                                accum_out=acc)
                    else:
                        nc.scalar.activation(interior, xqt[(s, cb, hh)][:],
                                             ACT_COPY, accum_out=acc)

        def se_chain(s):
            with nc.named_scope(f"se{s}"):
                z_ps = pstp.tile([128, 1], F32, tag="pt", name=f"z{s}")
                for i, (blk, col) in enumerate(zcols):
                    nc.tensor.matmul(z_ps[0:HID, :], prm["fc1t"][:, blk, :],
                                     pooled[s][:, col:col + 1],
                                     start=(i == 0), stop=(i == len(zcols) - 1))
                h_ext = smallp.tile([128, 1], F32, tag="hext", name=f"hext{s}")
                nc.vector.memset(h_ext[:], 1.0)  # row 65 stays 1.0 (bias row)
                # relu(z/4096): mean folded via scale (relu is scale-invariant)
                nc.scalar.activation(h_ext[0:HID, :], z_ps[0:HID, :], ACT_RELU,
                                     scale=1.0 / (H * W))
                y_ps = pstp.tile([128, K * 2], F32, tag="pt", name=f"y{s}")
                for c in range(K * 2):
                    nc.tensor.matmul(y_ps[:, c:c + 1],
                                     prm["fc2t"][0:HID + 1, c * 128:(c + 1) * 128],
                                     h_ext[0:HID + 1, :], start=True, stop=True)
                e = smallp.tile([128, K, 2], F32, tag="e", name=f"e{s}")
                nc.scalar.activation(e[:].rearrange("p a b -> p (a b)"),
                                     y_ps[:], ACT_EXP, scale=1.0 / TEMP)
                ssum = smallp.tile([128, 2], F32, tag="ssum", name=f"ssum{s}")
                er = e[:].rearrange("p k o -> p o k")
                nc.vector.tensor_reduce(ssum[:], er, mybir.AxisListType.X, ADD)
                rinv = smallp.tile([128, 2], F32, tag="rinv", name=f"rinv{s}")
                nc.vector.reciprocal(rinv[:], ssum[:])
                # softmax denominator folded into a final agg scale (mix_one)
                return e, rinv

        # ---- W load + cast ----------------------------------------------
        wb = [wbank.tile([128, K, C, NOFF], BF16, name=f"wb{ob}")
              for ob in range(2)]

        wstg = {}

        def load_w_dma(ob, cb):
            # ci-half-major chunks so the mix for ci-block 0 can start
            # while ci-block 1 is still in flight on the DMA ring
            with nc.named_scope(f"wload{ob}"):
                for k in range(K):
                    wst = wstage.tile([128, CF // 2], F32, tag="wst")
                    wstg[(ob, cb, k)] = wst
                    nc.sync.dma_start(
                        wst[:],
                        w_d[k, ob * 128:(ob + 1) * 128,
                            cb * 128:(cb + 1) * 128].rearrange(
                                "p c a b -> p (c a b)"))

        def w_cast(ob, cb, dve=False):
            with nc.named_scope(f"wcast{ob}"):
                for k in range(K):
                    dst = wb[ob][:, k, cb * 128:(cb + 1) * 128, :].rearrange(
                        "p c o -> p (c o)")
                    if dve:
                        nc.vector.tensor_copy(dst, wstg[(ob, cb, k)][:])
                    else:
                        nc.scalar.copy(dst, wstg[(ob, cb, k)][:])

        def load_w(ob, cbs=(0, 1)):
            for cb in cbs:
                load_w_dma(ob, cb)
                w_cast(ob, cb)

        # ---- mix + transposes + Wt --------------------------------------
        diag = {}

        def mix_pe(s, ob, agg, cbs):
            """agg[ob] = sum_k diag(e_k) @ W_k on the (startup-idle) PE,
            with the softmax 1/sum folded into the psum->sbuf copy."""
            e, rinv = se[s]
            with nc.named_scope(f"mixpe{s}_{ob}"):
                if (s, ob) not in diag:
                    dg = smallp.tile([128, K, 128], BF16, tag="diag",
                                     name=f"dg{s}_{ob}")
                    for k in range(K):
                        nc.vector.tensor_scalar_mul(dg[:, k, :],
                                                    prm["ident"][:],
                                                    e[:, k, ob:ob + 1])
                    diag[(s, ob)] = dg
                dg = diag[(s, ob)]
                af = agg[ob][:].rearrange("p c o -> p (c o)")
                for cb in cbs:
                    wf = wb[ob][:, :, cb * 128:(cb + 1) * 128, :].rearrange(
                        "p k c o -> p k (c o)")
                    for ci, (c0, cw) in enumerate(
                            ((0, 512), (512, 512), (1024, 128))):
                        ps = pstp.tile([128, 512], F32, tag="pt",
                                       name=f"mx{s}_{ob}_{cb}_{ci}")
                        dst = ps[:, 0:cw]
                        for k in range(K):
                            nc.tensor.matmul(dst, dg[:, k, :],
                                             wf[:, k, c0:c0 + cw],
                                             start=(k == 0), stop=(k == K - 1))
                        nc.scalar.activation(
                            af[:, cb * 1152 + c0:cb * 1152 + c0 + cw], dst,
                            ACT_COPY, scale=rinv[:, ob:ob + 1])

        def mix_one(s, ob, cb, agg, eng=None):
            eng = eng or nc.vector
            e, rinv = se[s]
            asl = agg[ob][:, cb * 128:(cb + 1) * 128, :]
            with nc.named_scope(f"mix{s}_{ob}"):
                eng.tensor_scalar_mul(
                    asl, wb[ob][:, 0, cb * 128:(cb + 1) * 128, :],
                    e[:, 0, ob:ob + 1])
                for k in range(1, K):
                    eng.scalar_tensor_tensor(
                        asl, wb[ob][:, k, cb * 128:(cb + 1) * 128, :],
                        e[:, k, ob:ob + 1], asl, MULT, ADD)
                eng.tensor_scalar_mul(asl, asl, rinv[:, ob:ob + 1])

        def transp(s, ob, agg, aggt, copy_eng=None, cbs=(0, 1)):
            copy = copy_eng or nc.scalar.copy
            with nc.named_scope(f"transp{s}_{ob}"):
                for cb in cbs:
                    for gi, (o0, o1) in enumerate(TGROUPS):
                        n = o1 - o0
                        pt = pstp.tile([128, 4, 128], BF16, tag="pt",
                                       name=f"pt{s}_{ob}_{cb}_{gi}")
                        for oi in range(n):
                            nc.tensor.transpose(
                                pt[:, oi, :],
                                agg[ob][:, cb * 128:(cb + 1) * 128, o0 + oi],
                                prm["ident"][:])
                        src = pt[:, 0:n, :]
                        dst = aggt[cb][:, o0:o1, ob * 128:(ob + 1) * 128]
                        copy(dst, src)

        def wt_build(s, aggt, wt, ob, cbs=(0, 1)):
            """wt[(cb,ob)] = [128, 2, 3, 128]: j1 = s0+s1+s2, j2 = s0-s1+s2
            (kh-planes of aggT); 1/2 factor applied at the M copy."""
            obs = slice(ob * 128, (ob + 1) * 128)
            for cb in cbs:
                t = wtp.tile([128, 2, KK, 128], BF16, tag="wt",
                             name=f"wt{s}_{cb}_{ob}")
                tmp = smallp.tile([128, KK, 128], BF16, tag="wtmp",
                                  name=f"wtmp{s}_{cb}_{ob}")
                a = aggt[cb]
                with nc.named_scope(f"wt{s}"):
                    nc.vector.tensor_tensor(tmp[:], a[:, 0:3, obs],
                                            a[:, 6:9, obs], ADD)
                    nc.vector.tensor_tensor(t[:, 0], tmp[:], a[:, 3:6, obs],
                                            ADD)
                    nc.vector.tensor_tensor(t[:, 1], tmp[:], a[:, 3:6, obs],
                                            SUB)
                wt[(cb, ob)] = t

        # ---- Winograd U build -------------------------------------------
        def u_pair(s, cb, hf, pair, ud):
            """one U pair tile for (s, cb, half): A = (u0, u3), B = (u1, u2);
            [128, 2, NT/2, UW] bf16, cols = xb cols 1..66."""
            t = xb[(s, cb)]

            def d(m):
                r0 = m + 32 * hf
                return t[:, r0:r0 + NT - 1:2, 1:1 + UW]

            nm = "ab"[pair]
            with nc.named_scope(f"u{s}"):
                u = up.tile([128, 2, NT // 2, UW], BF16, tag="u",
                            name=f"u{nm}{s}_{cb}_{hf}")
                if pair == 0:
                    nc.vector.tensor_tensor(u[:, 0], d(0), d(2), SUB)  # u0
                    nc.vector.tensor_tensor(u[:, 1], d(1), d(3), SUB)  # u3
                else:
                    nc.vector.tensor_tensor(u[:, 0], d(1), d(2), ADD)  # u1
                    nc.vector.tensor_tensor(u[:, 1], d(2), d(1), SUB)  # u2
            ud[(cb, hf)] = u

        # ---- conv via winograd GEMMs ------------------------------------
        def conv(s, aggt, wt, ua, ub, fillers, defer=None):
            out_hw = out_d[s].rearrange("o a b -> o (a b)")

            def lhsT(pair, jj, cb, kw, ob):
                obs = slice(ob * 128, (ob + 1) * 128)
                if pair == 0:  # (j0, j3) -> kh plane 0 / 2 of aggT
                    return aggt[cb][:, (0 if jj == 0 else 6) + kw, obs]
                return wt[(cb, ob)][:, jj, kw, :]

            def mms(ps, ob, pair, tc, cb):
                usrc = ua if pair == 0 else ub
                tl = (tc * TCH) % 16
                for jj in range(2):
                    for kw in range(KK):
                        nc.tensor.matmul(
                            ps[:, jj, :],
                            lhsT(pair, jj, cb, kw, ob),
                            usrc[(cb, tc // 2)][:, jj, tl:tl + TCH, kw:kw + W],
                            start=(cb == 0 and kw == 0),
                            stop=(cb == 1 and kw == KK - 1))

            def m_copy(ps, ob, pair, tc, mtile):
                tl = (tc * TCH) % 16
                dst = mtile[:, 2 * pair:2 * pair + 2, tl:tl + TCH, :]
                src = ps[:].rearrange("p a (b c) -> p a b c", b=TCH)
                if pair == 0:
                    nc.scalar.copy(dst, src)
                else:  # fold the F(2,3) 1/2 into the copy
                    nc.scalar.activation(dst, src, ACT_COPY, scale=0.5)

            def m_chunk(ob, pair, tc, mtile):
                ps = pscp.tile([128, 2, 512], F32, tag="conv",
                               name=f"ps{s}_{ob}_{pair}_{tc}")
                for cb in range(2):
                    mms(ps, ob, pair, tc, cb)
                m_copy(ps, ob, pair, tc, mtile)

            def inverse(ob, half, q, mtile, eng=None):
                eng = eng or nc.vector
                # m slots: 0=j0, 1=j3, 2=j1, 3=j2
                # even row 2t   = M0+M1+M2 ; odd row 2t+1 = M1-M2-M3
                st = ostp.tile([128, 16, W], F32, tag="ost", bufs=2,
                               name=f"st{s}_{ob}_{half}_{q}")
                i1 = invp.tile([128, 8, W], BF16, tag="i1",
                               name=f"i1{s}_{ob}_{half}_{q}")
                i2 = invp.tile([128, 8, W], BF16, tag="i2",
                               name=f"i2{s}_{ob}_{half}_{q}")
                tq = slice(8 * q, 8 * q + 8)
                with nc.named_scope(f"inv{s}_{ob}"):
                    eng.tensor_tensor(i1[:], mtile[:, 2, tq],
                                      mtile[:, 3, tq], ADD)
                    eng.tensor_tensor(st[:, 0:16:2, :], i1[:],
                                      mtile[:, 0, tq], ADD)
                    eng.tensor_tensor(i2[:], mtile[:, 2, tq],
                                      mtile[:, 3, tq], SUB)
                    eng.tensor_tensor(st[:, 1:16:2, :], i2[:],
                                      mtile[:, 1, tq], SUB)
                r0 = half * 32 + 16 * q
                nc.sync.dma_start(
                    out_hw[ob * 128:(ob + 1) * 128, r0 * W:(r0 + 16) * W],
                    st[:].rearrange("p a b -> p (a b)"))

            def tail(mtile):
                # final half-block's B chunks split in four 4-tile pieces to
                # shorten the copy -> inverse -> DMA drain after the last mm
                for sub in range(4):
                    ps = pscp.tile([128, 2, 256], F32, tag="conv",
                                   name=f"pstail{sub}")
                    tl = 4 * sub
                    for jj in range(2):
                        for cb in range(2):
                            for kw in range(KK):
                                nc.tensor.matmul(
                                    ps[:, jj, :],
                                    lhsT(1, jj, cb, kw, 1),
                                    ub[(cb, 1)][:, jj, tl:tl + 4, kw:kw + W],
                                    start=(cb == 0 and kw == 0),
                                    stop=(cb == 1 and kw == KK - 1))
                    dst = mtile[:, 2:4, tl:tl + 4, :]
                    nc.scalar.activation(
                        dst, ps[:].rearrange("p a (b c) -> p a b c", b=4),
                        ACT_COPY, scale=0.5)
                    st = ostp.tile([128, 8, W], F32, tag="ost8", bufs=3,
                                   name=f"sttail{sub}")
                    i1 = invp.tile([128, 4, W], BF16, tag="i1",
                                   name=f"i1tail{sub}")
                    i2 = invp.tile([128, 4, W], BF16, tag="i2",
                                   name=f"i2tail{sub}")
                    tq = slice(tl, tl + 4)
                    nc.vector.tensor_tensor(i1[:], mtile[:, 2, tq],
                                            mtile[:, 3, tq], ADD)
                    nc.vector.tensor_tensor(st[:, 0:8:2, :], i1[:],
                                            mtile[:, 0, tq], ADD)
                    nc.vector.tensor_tensor(i2[:], mtile[:, 2, tq],
                                            mtile[:, 3, tq], SUB)
                    nc.vector.tensor_tensor(st[:, 1:8:2, :], i2[:],
                                            mtile[:, 1, tq], SUB)
                    r0 = 32 + 8 * sub
                    nc.sync.dma_start(
                        out_hw[128:256, r0 * W:(r0 + 8) * W],
                        st[:].rearrange("p a b -> p (a b)"))

            def inv_or_defer(ob, hf, q, mtile):
                if defer is not None and ob == 1:
                    defer.append(lambda ob=ob, hf=hf, q=q, m=mtile:
                                 inverse(ob, hf, q, m, eng=nc.gpsimd))
                else:
                    inverse(ob, hf, q, mtile)

            with nc.named_scope(f"conv{s}"):
                for ob in range(2):
                    def point(i, ob=ob):
                        f = fillers.get((ob, point.hf, i))
                        if f is not None:
                            f()
                    mt = [mp.tile([128, 4, 16, W], BF16, tag="m",
                                  name=f"m{s}_{ob}_{hf}") for hf in range(2)]
                    for hf in range(2):
                        point.hf = hf
                        t0, t1 = 2 * hf, 2 * hf + 1
                        if hf == 0:
                            # stream all ci-block-0 matmuls before ci-block
                            # 1's weights/U have finished
                            psa0 = pscp.tile([128, 2, 512], F32, tag="conv",
                                             name=f"psa{s}_{ob}_0")
                            psa1 = pscp.tile([128, 2, 512], F32, tag="conv",
                                             name=f"psa{s}_{ob}_1")
                            psb0 = pscp.tile([128, 2, 512], F32, tag="conv",
                                             name=f"psb{s}_{ob}_0")
                            mms(psa0, ob, 0, t0, 0)
                            point(0)
                            mms(psa1, ob, 0, t1, 0)
                            point(1)
                            mms(psb0, ob, 1, t0, 0)
                            point(2)
                            mms(psa0, ob, 0, t0, 1)
                            m_copy(psa0, ob, 0, t0, mt[hf])
                            point(3)
                            mms(psa1, ob, 0, t1, 1)
                            m_copy(psa1, ob, 0, t1, mt[hf])
                            point(4)
                            mms(psb0, ob, 1, t0, 1)
                            m_copy(psb0, ob, 1, t0, mt[hf])
                            point(5)
                            inv_or_defer(ob, hf, 0, mt[hf])
                            point(6)
                            m_chunk(ob, 1, t1, mt[hf])
                            point(7)
                            inv_or_defer(ob, hf, 1, mt[hf])
                            point(8)
                        else:
                            m_chunk(ob, 0, t0, mt[hf])
                            point(0)
                            m_chunk(ob, 0, t1, mt[hf])
                            point(1)
                            if s == 1 and ob == 1:
                                point(2)
                                point(3)
                                tail(mt[hf])
                                continue
                            m_chunk(ob, 1, t0, mt[hf])
                            point(2)
                            inv_or_defer(ob, hf, 0, mt[hf])
                            point(3)
                            m_chunk(ob, 1, t1, mt[hf])
                            point(4)
                            inv_or_defer(ob, hf, 1, mt[hf])
                            point(5)

        # ---- emission ----------------------------------------------------
        agg0 = [aggp.tile([128, C, NOFF], BF16, tag="agg", name=f"agg0_{ob}")
                for ob in range(2)]
        aggt0 = [aggtp.tile([128, NOFF, O], BF16, tag="aggt",
                            name=f"aggt0_{cb}") for cb in range(2)]
        ua0, ub0, wt0 = {}, {}, {}
        # DMA queue order: W0a, x0c1, x0c0, fc-params, W0b | W1a, W1b,
        # x1c0, x1c1 | conv0 outs
        load_w(0, (0,))
        xload_dma(0, 1)
        xcast(0, 1)
        params()
        xload_dma(0, 0)
        xcast(0, 0)
        se.append(se_chain(0))
        load_w_dma(0, 1)
        w_cast(0, 1)
        u_pair(0, 1, 0, 0, ua0)
        u_pair(0, 1, 0, 1, ub0)
        u_pair(0, 0, 0, 0, ua0)
        mix_pe(0, 0, agg0, (0,))
        u_pair(0, 0, 0, 1, ub0)
        transp(0, 0, agg0, aggt0, cbs=(0,),
               copy_eng=nc.vector.tensor_copy)
        wt_build(0, aggt0, wt0, 0, (0,))
        u_pair(0, 0, 1, 0, ua0)
        u_pair(0, 1, 1, 0, ua0)
        u_pair(0, 0, 1, 1, ub0)
        u_pair(0, 1, 1, 1, ub0)
        # sample-1 DMAs enqueued now; their casts run as conv(0) fillers
        load_w_dma(1, 0)
        load_w_dma(1, 1)
        xload_dma(1, 0)
        xload_dma(1, 1)

        # sample-1 prep emitted as fillers inside conv(0) so the in-order
        # DVE/ACT/PE queues interleave it with sample-0's conv stream;
        # keys are (ob, hf, position) emission points of conv()
        agg1 = [aggp.tile([128, C, NOFF], BF16, tag="agg", name=f"agg1_{ob}")
                for ob in range(2)]
        aggt1 = [aggtp.tile([128, NOFF, O], BF16, tag="aggt",
                            name=f"aggt1_{cb}") for cb in range(2)]
        ua1, ub1, wt1 = {}, {}, {}
        f0 = {
            (0, 0, 2): lambda: (mix_pe(0, 0, agg0, (1,)),
                                transp(0, 0, agg0, aggt0, cbs=(1,),
                                       copy_eng=nc.vector.tensor_copy)),
            (0, 0, 4): lambda: wt_build(0, aggt0, wt0, 0, (1,)),
            (0, 1, 0): lambda: w_cast(1, 0),
            (0, 1, 1): lambda: mix_one(0, 1, 0, agg0),
            (0, 1, 2): lambda: w_cast(1, 1),
            (0, 1, 4): lambda: mix_one(0, 1, 1, agg0),
            (0, 1, 5): lambda: transp(0, 1, agg0, aggt0, cbs=(0,)),
            (1, 0, 0): lambda: wt_build(0, aggt0, wt0, 1, (0,)),
            (1, 0, 1): lambda: xcast(1, 0, dve=True),
            (1, 0, 2): lambda: transp(0, 1, agg0, aggt0, cbs=(1,)),
            (1, 0, 4): lambda: (wt_build(0, aggt0, wt0, 1, (1,)),
                                u_pair(1, 0, 0, 0, ua1),
                                u_pair(1, 0, 0, 1, ub1)),
            (1, 0, 5): lambda: xcast(1, 1, dve=True),
            (1, 0, 6): lambda: se.append(se_chain(1)),
            (1, 1, 0): lambda: (u_pair(1, 1, 0, 0, ua1),
                                u_pair(1, 1, 0, 1, ub1)),
            (1, 1, 1): lambda: (mix_one(1, 0, 0, agg1),
                                mix_one(1, 0, 1, agg1)),
            (1, 1, 3): lambda: (mix_one(1, 1, 0, agg1),
                                mix_one(1, 1, 1, agg1)),
            (1, 1, 5): lambda: (u_pair(1, 0, 1, 0, ua1),
                                u_pair(1, 1, 1, 0, ua1)),
        }
        deferred = []
        conv(0, aggt0, wt0, ua0, ub0, f0, defer=deferred)
        transp(1, 0, agg1, aggt1)
        f1 = {
            (0, 0, 0): lambda: wt_build(1, aggt1, wt1, 0),
            (0, 0, 2): lambda: (u_pair(1, 0, 1, 1, ub1),
                                u_pair(1, 1, 1, 1, ub1)),
            (0, 0, 4): lambda: deferred[0](),
            (0, 0, 6): lambda: transp(1, 1, agg1, aggt1),
            (0, 0, 8): lambda: deferred[1](),
            (0, 1, 0): lambda: wt_build(1, aggt1, wt1, 1),
            (0, 1, 2): lambda: deferred[2](),
            (0, 1, 4): lambda: deferred[3](),
        }
        conv(1, aggt1, wt1, ua1, ub1, f1)


_NC_CACHE = None


def _get_nc():
    global _NC_CACHE
    if _NC_CACHE is None:
        _NC_CACHE = build_kernel()
    return _NC_CACHE


def make_in_maps(x, fc1_w, fc2_w, fc2_b, weight):
    x = np.ascontiguousarray(x, dtype=np.float32)
    shared = {
        "fc1_w": np.ascontiguousarray(fc1_w, dtype=np.float32),
        "fc2_w": np.ascontiguousarray(fc2_w, dtype=np.float32),
        "fc2_b": np.ascontiguousarray(fc2_b, dtype=np.float32),
        "weight": np.ascontiguousarray(weight, dtype=np.float32),
    }
    return [{"x": x[c * BS:(c + 1) * BS], **shared} for c in range(N_CORES)]


def kernel(x, fc1_w, fc2_w, fc2_b, weight):
    import time
    nc = _get_nc()
    in_maps = make_in_maps(x, fc1_w, fc2_w, fc2_b, weight)
    res = None
    for attempt in range(3):
        try:
            res = run_bass_kernel_spmd(nc, in_maps,
                                       core_ids=list(range(N_CORES)))
            break
        except Exception:
            # transient device wedge (NRT_EXEC_UNIT_UNRECOVERABLE); the
            # axon terminal recovers after a short wait
            if attempt == 2:
                raise
            time.sleep(60 * (attempt + 1))
    return np.concatenate([res.results[c]["out"] for c in range(N_CORES)],
                          axis=0).astype(np.float32)
